# revision 8
# baseline (speedup 1.0000x reference)
"""Trainium2 Bass kernel for nn_ModelX_47004122088026 (8 NeuronCores).

Strategy (tensor-parallel, feature/col-split across 8 cores):
- All activations kept transposed [feature, seq] (feature on partitions).
- Every Linear is column-split: core c computes its 256 output features,
  then a bf16 AllGather rebuilds the full activation for the next layer.
- Attention is head-split (2 heads/core, head_dim=128).
- Algebraic restructuring (exact):
  * The genetic population is 100 identical copies of `meta`, so the
    selection net + top_k are no-ops: best == meta. The crossover mean
    collapses to genetic = mut + (cnt/50)*(meta-mut) with
    cnt = sum_k (u_k < 0.8), computed on-device from crossover_u.
  * The architecture_optimizer chain (4 bias-linears, no nonlinearity)
    folds into one matrix W_ao (host fold) and bias b_ao.
  * prior_network(0) contribution folds into po_b1 (host).
- Matmuls in bf16 (fp32 PSUM accumulation); softmax/LN internals fp32.
"""
import numpy as np
import ml_dtypes

import concourse.bacc as bacc
import concourse.mybir as mybir
import concourse.tile as tile
from concourse.bass import ds, ts
from concourse.bass_utils import run_bass_kernel_spmd
from concourse.masks import make_identity

F32 = mybir.dt.float32
BF16 = mybir.dt.bfloat16
AF = mybir.ActivationFunctionType
ALU = mybir.AluOpType
AX = mybir.AxisListType

NCORES = 8
S = 512            # sequence
H = 2048           # hidden
NC = H // NCORES   # 256 features per core
NH, HD = 16, 128
HPC = NH // NCORES  # 2 heads per core
V = 50000
VC = V // NCORES   # 6250 vocab cols per core
KPOP = 50
CROSS = 0.8
ISQD = float(1.0 / np.sqrt(128.0))
RG = [list(range(NCORES))]

BF = ml_dtypes.bfloat16



_TC = [0]


def mktile(pool, shape, dtype, tag, bufs=1, **kw):
    _TC[0] += 1
    return pool.tile(shape, dtype, tag=tag, bufs=bufs, name=f"{tag}_{_TC[0]}", **kw)

def build_program(debug=False):
    nc = bacc.Bacc("TRN2", target_bir_lowering=False, debug=False,
                   num_devices=NCORES)

    def din(name, shape, dt=BF16):
        return nc.dram_tensor(name, shape, dt, kind="ExternalInput")

    xT_d = din("xT", [H, S])
    u_d = din("u", [KPOP, S, NC], F32)
    lnm_d = din("ln_mask", [2 * NCORES, 2], F32)
    qo_w1_d = din("qo_w1", [HD, 2 * HD])
    qo_w2_d = din("qo_w2", [2 * HD, HD])
    qo_b1_d = din("qo_b1", [2 * HD], F32)
    qo_b2_d = din("qo_b2", [HD], F32)
    wq_d = din("wq", [H, NC]); wk_d = din("wk", [H, NC]); wv_d = din("wv", [H, NC])
    wo_d = din("wo", [H, NC])
    ml_w1_d = din("ml_w1", [H, 2 * NC]); ml_b1_d = din("ml_b1", [2 * NC], F32)
    ml_w2_d = din("ml_w2", [2 * H, NC]); ml_b2_d = din("ml_b2", [NC], F32)
    ml_g_d = din("ml_g", [NC], F32); ml_bt_d = din("ml_bt", [NC], F32)
    ml_aw_d = din("ml_aw", [H, NC]); ml_ab_d = din("ml_ab", [NC], F32)
    mut_w1_d = din("mut_w1", [H, NC]); mut_b1_d = din("mut_b1", [NC], F32)
    mut_w2_d = din("mut_w2", [H, NC]); mut_b2_d = din("mut_b2", [NC], F32)
    ev_w1_d = din("ev_w1", [H, 2 * NC]); ev_b1_d = din("ev_b1", [2 * NC], F32)
    ev_w2_d = din("ev_w2", [2 * H, 2 * NC]); ev_b2_d = din("ev_b2", [2 * NC], F32)
    ev_w3_d = din("ev_w3", [2 * H, NC]); ev_b3_d = din("ev_b3", [NC], F32)
    wao_d = din("wao", [H, NC]); bao_d = din("bao", [NC], F32)
    lk_w1_d = din("lk_w1", [H, NC]); lk_b1_d = din("lk_b1", [NC], F32)
    lk_w2_d = din("lk_w2", [H, NC]); lk_b2_d = din("lk_b2", [NC], F32)
    po_w1_d = din("po_w1", [H, NC]); po_b1_d = din("po_b1", [NC], F32)
    po_w2_d = din("po_w2", [H, NC]); po_b2_d = din("po_b2", [NC], F32)
    int_w1_d = din("int_w1", [5 * H, 2 * NC]); int_b1_d = din("int_b1", [2 * NC], F32)
    int_w2_d = din("int_w2", [2 * H, NC]); int_b2_d = din("int_b2", [NC], F32)
    int_g_d = din("int_g", [NC], F32); int_bt_d = din("int_bt", [NC], F32)
    out_w_d = din("out_w", [H, VC]); out_b_d = din("out_b", [VC], F32)

    out_d = nc.dram_tensor("out", [S, VC], F32, kind="ExternalOutput")

    with tile.TileContext(nc) as tc:
        with (
            tc.tile_pool(name="cst", bufs=1) as cst,
            tc.tile_pool(name="xp", bufs=1) as xp,
            tc.tile_pool(name="wp", bufs=2) as wp,
            tc.tile_pool(name="fp", bufs=3) as fp,
            tc.tile_pool(name="loc", bufs=1) as loc,
            tc.tile_pool(name="ps", bufs=1, space="PSUM") as ps,
            tc.tile_pool(name="dram", bufs=1, space="DRAM") as dram,
        ):
            # ---------- constants ----------
            ident_bf = mktile(cst, [128, 128], BF16, tag="ident_bf")
            make_identity(nc, ident_bf)
            ident_f32 = mktile(cst, [128, 128], F32, tag="ident_f32")
            make_identity(nc, ident_f32)
            ones_col = mktile(cst, [128, 1], F32, tag="ones_col")
            nc.vector.memset(ones_col[:], 1.0)
            ones_row = mktile(cst, [1, 128], F32, tag="ones_row")
            nc.vector.memset(ones_row[:], 1.0)
            eps_t = mktile(cst, [1, 1], F32, tag="eps")
            nc.vector.memset(eps_t[:], 1e-5)
            lnm = mktile(cst, [2 * NCORES, 2], F32, tag="lnm")
            nc.sync.dma_start(out=lnm[:], in_=lnm_d[:])

            def bias_tile(dram_t, n, tag):
                t = mktile(cst, [128, max(1, n // 128)], F32, tag=tag)
                nc.sync.dma_start(out=t[:],
                                  in_=dram_t[:].rearrange("(t p) -> p t", p=128))
                return t

            b_qo1 = bias_tile(qo_b1_d, 2 * HD, "b_qo1")
            b_qo2 = bias_tile(qo_b2_d, HD, "b_qo2")
            b_ml1 = bias_tile(ml_b1_d, 2 * NC, "b_ml1")
            b_ml2 = bias_tile(ml_b2_d, NC, "b_ml2")
            g_ml = bias_tile(ml_g_d, NC, "g_ml")
            bt_ml = bias_tile(ml_bt_d, NC, "bt_ml")
            b_mla = bias_tile(ml_ab_d, NC, "b_mla")
            b_mu1 = bias_tile(mut_b1_d, NC, "b_mu1")
            b_mu2 = bias_tile(mut_b2_d, NC, "b_mu2")
            b_ev1 = bias_tile(ev_b1_d, 2 * NC, "b_ev1")
            b_ev2 = bias_tile(ev_b2_d, 2 * NC, "b_ev2")
            b_ev3 = bias_tile(ev_b3_d, NC, "b_ev3")
            b_ao = bias_tile(bao_d, NC, "b_ao")
            b_lk1 = bias_tile(lk_b1_d, NC, "b_lk1")
            b_lk2 = bias_tile(lk_b2_d, NC, "b_lk2")
            b_po1 = bias_tile(po_b1_d, NC, "b_po1")
            b_po2 = bias_tile(po_b2_d, NC, "b_po2")
            b_i1 = bias_tile(int_b1_d, 2 * NC, "b_i1")
            b_i2 = bias_tile(int_b2_d, NC, "b_i2")
            g_i = bias_tile(int_g_d, NC, "g_i")
            bt_i = bias_tile(int_bt_d, NC, "bt_i")

            xT_sb = mktile(xp, [128, 16, S], BF16, tag="xresid")
            nc.sync.dma_start(out=xT_sb[:],
                              in_=xT_d[:].rearrange("(kt p) s -> p kt s", p=128))

            # ---------- helpers ----------
            def ag(shard_tiles, ntiles, name):
                shard_d = mktile(dram, [ntiles * 128, S], BF16, tag=f"sh_{name}")
                for i, t in enumerate(shard_tiles):
                    nc.sync.dma_start(out=shard_d[ts(i, 128), :], in_=t)
                full_d = mktile(dram, [ntiles * 128 * NCORES, S], BF16,
                                   addr_space="Shared", tag=f"fl_{name}")
                nc.gpsimd.collective_compute(
                    "AllGather", ALU.bypass, replica_groups=RG,
                    ins=[shard_d[:]], outs=[full_d[:]])
                return full_d

            def rhs_stream(full_d, kts):
                slabs = {}
                def rhs(kt):
                    s0 = (kt // 16) * 16
                    if s0 not in slabs:
                        sl = min(16, kts - s0)
                        t = mktile(fp, [128, sl, S], BF16, tag="full")
                        nc.sync.dma_start(
                            out=t[:],
                            in_=full_d[ds(s0 * 128, sl * 128), :].rearrange(
                                "(kt p) s -> p kt s", p=128))
                        slabs[s0] = t
                    return slabs[s0][:, kt - s0, :]
                return rhs

            def layer(w_d, Kdim, ncols, rhs_fn, evict_fn):
                kts, nmt = Kdim // 128, ncols // 128
                psums = [mktile(ps, [128, S], F32, tag=f"mm{mt}")
                         for mt in range(nmt)]
                for s0 in range(0, kts, 16):
                    sl = min(16, kts - s0)
                    wt = mktile(wp, [128, sl, ncols], BF16, tag="w")
                    nc.sync.dma_start(
                        out=wt[:],
                        in_=w_d[ds(s0 * 128, sl * 128), :].rearrange(
                            "(kt p) n -> p kt n", p=128))
                    for kt in range(sl):
                        rhs = rhs_fn(s0 + kt)
                        for mt in range(nmt):
                            nc.tensor.matmul(
                                psums[mt][:], wt[:, kt, ts(mt, 128)], rhs,
                                start=(s0 + kt == 0),
                                stop=(s0 + kt == kts - 1))
                for mt in range(nmt):
                    evict_fn(mt, psums[mt])

            def sb_tiles(t, nmt):
                return [t[:, mt, :] for mt in range(nmt)]

            def layer_ag(name, w_d, Kdim, ncols, rhs_fn, bias, act="copy"):
                nmt = ncols // 128
                ot = mktile(loc, [128, nmt, S], BF16, tag="oshard", bufs=2)
                fn = AF.Gelu if act == "gelu" else AF.Identity
                def ev(mt, psum):
                    nc.scalar.activation(ot[:, mt, :], psum[:], fn,
                                         bias=bias[:, mt:mt + 1])
                layer(w_d, Kdim, ncols, rhs_fn, ev)
                return ag(sb_tiles(ot, nmt), nmt, name)

            def ln_apply(name, pss, loc_f32, nmt, gam, bet, out_bf):
                sq = mktile(loc, [128, nmt, S], F32, tag="lnsq")
                for mt in range(nmt):
                    nc.scalar.activation(sq[:, mt, :], loc_f32[:, mt, :],
                                         AF.Square)
                ps1 = mktile(pss, [1, S], F32, tag="st1")
                ps2 = mktile(pss, [1, S], F32, tag="st2")
                for mt in range(nmt):
                    nc.tensor.matmul(ps1[:], ones_col[:], loc_f32[:, mt, :],
                                     start=(mt == 0), stop=(mt == nmt - 1))
                for mt in range(nmt):
                    nc.tensor.matmul(ps2[:], ones_col[:], sq[:, mt, :],
                                     start=(mt == 0), stop=(mt == nmt - 1))
                st1_sb = mktile(loc, [1, S], F32, tag="lnst1")
                st2_sb = mktile(loc, [1, S], F32, tag="lnst2")
                nc.scalar.activation(st1_sb[:], ps1[:], AF.Copy)
                nc.scalar.activation(st2_sb[:], ps2[:], AF.Copy)
                st_sh = mktile(dram, [2, S], F32, tag=f"stsh_{name}")
                nc.sync.dma_start(out=st_sh[0:1, :], in_=st1_sb[:])
                nc.sync.dma_start(out=st_sh[1:2, :], in_=st2_sb[:])
                st_fl = mktile(dram, [2 * NCORES, S], F32, addr_space="Shared",
                                  tag=f"stfl_{name}")
                nc.gpsimd.collective_compute(
                    "AllGather", ALU.bypass, replica_groups=RG,
                    ins=[st_sh[:]], outs=[st_fl[:]])
                st_all = mktile(loc, [2 * NCORES, S], F32, tag="lnsta")
                nc.sync.dma_start(out=st_all[:], in_=st_fl[:])
                psr1 = mktile(pss, [1, S], F32, tag="st1")
                psr2 = mktile(pss, [1, S], F32, tag="st2")
                nc.tensor.matmul(psr1[:], lnm[:, 0:1], st_all[:], start=True,
                                 stop=True)
                nc.tensor.matmul(psr2[:], lnm[:, 1:2], st_all[:], start=True,
                                 stop=True)
                m = mktile(loc, [1, S], F32, tag="ln_m")
                ex2 = mktile(loc, [1, S], F32, tag="ln_e")
                nc.vector.tensor_scalar_mul(m[:], psr1[:], 1.0 / H)
                nc.vector.tensor_scalar_mul(ex2[:], psr2[:], 1.0 / H)
                mm = mktile(loc, [1, S], F32, tag="ln_mm")
                nc.vector.tensor_mul(mm[:], m[:], m[:])
                var = mktile(loc, [1, S], F32, tag="ln_v")
                nc.vector.tensor_sub(var[:], ex2[:], mm[:])
                sstd = mktile(loc, [1, S], F32, tag="ln_sd")
                nc.scalar.activation(sstd[:], var[:], AF.Sqrt, bias=eps_t[:])
                rstd = mktile(loc, [1, S], F32, tag="ln_r")
                nc.vector.reciprocal(rstd[:], sstd[:])
                nm = mktile(loc, [1, S], F32, tag="ln_nm")
                nc.vector.tensor_scalar_mul(nm[:], m[:], -1.0)
                mb = mktile(loc, [1, S], F32, tag="ln_mb")
                nc.vector.tensor_mul(mb[:], nm[:], rstd[:])
                rstd_bc = mktile(pss, [128, S], F32, tag="bc1")
                mb_bc = mktile(pss, [128, S], F32, tag="bc2")
                nc.tensor.matmul(rstd_bc[:], ones_row[:], rstd[:],
                                 start=True, stop=True)
                nc.tensor.matmul(mb_bc[:], ones_row[:], mb[:],
                                 start=True, stop=True)
                for mt in range(nmt):
                    t1 = mktile(loc, [128, S], F32, tag="ln_t1")
                    nc.vector.tensor_mul(t1[:], loc_f32[:, mt, :], rstd_bc[:])
                    t2 = mktile(loc, [128, S], F32, tag="ln_t2")
                    nc.vector.tensor_add(t2[:], t1[:], mb_bc[:])
                    nc.vector.tensor_scalar(out_bf[:, mt, :], t2[:],
                                            gam[:, mt:mt + 1],
                                            bet[:, mt:mt + 1],
                                            ALU.mult, ALU.add)

            frT = mktile(loc, [128, 2, S], F32, tag="frT")
            ao_f32 = mktile(loc, [128, 2, S], F32, tag="ao_f32")
            ao_bf = mktile(loc, [128, 2, S], BF16, tag="ao_bf")

            # ============ phase: crossover-frac + attention ============
            with (
                tc.tile_pool(name="up", bufs=2) as up,
                tc.tile_pool(name="att", bufs=1) as att,
                tc.tile_pool(name="psa", bufs=2, space="PSUM") as psa,
            ):
                # crossover count, natural layout then PE-transpose
                for st in range(4):
                    acc = [mktile(up, [128, NC], F32, tag="acc_a"),
                           mktile(up, [128, NC], F32, tag="acc_b")]
                    for fc in range(8):  # feature chunks of 32
                        uch = mktile(up, [128, KPOP, 32], F32, tag="uch")
                        nc.sync.dma_start(
                            out=uch[:],
                            in_=u_d[:, ts(st, 128), ts(fc, 32)].rearrange(
                                "k p f -> p k f"))
                        fsl = ds(fc * 32, 32)
                        nc.vector.tensor_scalar(acc[0][:, fsl], uch[:, 0, :],
                                                CROSS, None, ALU.is_lt)
                        for k in range(1, KPOP):
                            nc.vector.scalar_tensor_tensor(
                                acc[k % 2][:, fsl], uch[:, k, :], CROSS,
                                acc[(k - 1) % 2][:, fsl], ALU.is_lt, ALU.add)
                    for ft in range(2):
                        tp = mktile(psa, [128, 128], F32, tag="aux")
                        nc.tensor.transpose(
                            tp[:], acc[(KPOP - 1) % 2][:, ts(ft, 128)],
                            ident_f32[:])
                        nc.scalar.activation(frT[:, ft, ts(st, 128)], tp[:],
                                             AF.Copy)

                # q/k/v projections (2 heads per core)
                qb = mktile(att, [128, HPC, S], BF16, tag="qb")
                kb = mktile(att, [128, HPC, S], BF16, tag="kb")
                vvb = mktile(att, [128, HPC, S], BF16, tag="vvb")

                def ev_bf(t):
                    def f(mt, psum):
                        nc.scalar.activation(t[:, mt, :], psum[:], AF.Copy)
                    return f
                layer(wq_d, H, NC, lambda kt: xT_sb[:, kt, :], ev_bf(qb))
                layer(wk_d, H, NC, lambda kt: xT_sb[:, kt, :], ev_bf(kb))
                layer(wv_d, H, NC, lambda kt: xT_sb[:, kt, :], ev_bf(vvb))

                vh = mktile(att, [128, HPC, 4, 128], BF16, tag="vh")
                for h in range(HPC):
                    for kt4 in range(4):
                        tp = mktile(psa, [128, 128], BF16, tag="attp")
                        nc.tensor.transpose(tp[:], vvb[:, h, ts(kt4, 128)],
                                            ident_bf[:])
                        nc.scalar.activation(vh[:, h, kt4, :], tp[:], AF.Copy)

                qo1_sb = mktile(att, [128, 2 * HD], BF16, tag="qo1")
                nc.sync.dma_start(out=qo1_sb[:], in_=qo_w1_d[:])
                qo2_sb = mktile(att, [128, 2, HD], BF16, tag="qo2")
                nc.sync.dma_start(
                    out=qo2_sb[:],
                    in_=qo_w2_d[:].rearrange("(kt p) n -> p kt n", p=128))

                def qopt(src, h, nm):
                    g = mktile(att, [128, 2, S], BF16, tag=f"qog_{nm}")
                    for mt in range(2):
                        y1 = mktile(psa, [128, S], F32, tag="aux")
                        nc.tensor.matmul(y1[:], qo1_sb[:, ts(mt, 128)],
                                         src[:, h, :], start=True, stop=True)
                        nc.scalar.activation(g[:, mt, :], y1[:], AF.Gelu,
                                             bias=b_qo1[:, mt:mt + 1])
                    y2 = mktile(psa, [128, S], F32, tag="aux")
                    for kt in range(2):
                        nc.tensor.matmul(y2[:], qo2_sb[:, kt, :], g[:, kt, :],
                                         start=(kt == 0), stop=(kt == 1))
                    o = mktile(att, [128, S], BF16, tag=f"qh_{nm}")
                    nc.scalar.activation(o[:], y2[:], AF.Identity,
                                         bias=b_qo2[:, 0:1])
                    return o

                avb = mktile(att, [128, HPC, S], BF16, tag="avb")
                for h in range(HPC):
                    qh = qopt(qb, h, f"q{h}")
                    kh = qopt(kb, h, f"k{h}")
                    attn = mktile(att, [128, 4, S], BF16, tag="attn", bufs=2)
                    for st in range(4):
                        sc = mktile(psa, [128, S], F32, tag="aux")
                        nc.tensor.matmul(sc[:], qh[:, ts(st, 128)], kh[:],
                                         start=True, stop=True)
                        rmax = mktile(att, [128, 1], F32, tag="rmax", bufs=2)
                        nc.vector.reduce_max(rmax[:], sc[:], AX.X)
                        nbias = mktile(att, [128, 1], F32, tag="nbias", bufs=2)
                        nc.vector.tensor_scalar_mul(nbias[:], rmax[:], -ISQD)
                        esc = mktile(att, [128, S], BF16, tag="esc", bufs=2)
                        rsum = mktile(att, [128, 1], F32, tag="rsum", bufs=2)
                        nc.scalar.activation(esc[:], sc[:], AF.Exp,
                                             bias=nbias[:], scale=ISQD,
                                             accum_out=rsum[:])
                        rinv = mktile(att, [128, 1], F32, tag="rinv", bufs=2)
                        nc.vector.reciprocal(rinv[:], rsum[:])
                        nc.vector.tensor_scalar_mul(attn[:, st, :], esc[:],
                                                    rinv[:])
                    attnT = mktile(att, [128, 4, S], BF16, tag="attnT", bufs=2)
                    for st in range(4):
                        for kt4 in range(4):
                            tp = mktile(psa, [128, 128], BF16, tag="attp")
                            nc.tensor.transpose(tp[:],
                                                attn[:, st, ts(kt4, 128)],
                                                ident_bf[:])
                            nc.scalar.activation(attnT[:, kt4, ts(st, 128)],
                                                 tp[:], AF.Copy)
                    av_ps = mktile(psa, [128, S], F32, tag="aux")
                    for kt4 in range(4):
                        nc.tensor.matmul(av_ps[:], vh[:, h, kt4, :],
                                         attnT[:, kt4, :],
                                         start=(kt4 == 0), stop=(kt4 == 3))
                    nc.scalar.activation(avb[:, h, :], av_ps[:], AF.Copy)

                avF = ag(sb_tiles(avb, HPC), HPC, "av")

                def ev_ao(mt, psum):
                    nc.scalar.activation(ao_f32[:, mt, :], psum[:], AF.Copy)
                    nc.vector.tensor_copy(ao_bf[:, mt, :], ao_f32[:, mt, :])
                layer(wo_d, H, NC, rhs_stream(avF, 16), ev_ao)
                aoF = ag(sb_tiles(ao_bf, 2), 2, "ao")

            # ============ meta ============
            g1F = layer_ag("g1", ml_w1_d, H, 2 * NC, rhs_stream(aoF, 16),
                           b_ml1, act="gelu")
            mf_f32 = mktile(loc, [128, 2, S], F32, tag="mf_f32")
            layer(ml_w2_d, 2 * H, NC, rhs_stream(g1F, 32),
                  lambda mt, psum: nc.scalar.activation(
                      mf_f32[:, mt, :], psum[:], AF.Identity,
                      bias=b_ml2[:, mt:mt + 1]))
            mfn_bf = mktile(loc, [128, 2, S], BF16, tag="mfn_bf")
            with tc.tile_pool(name="pml", bufs=1, space="PSUM") as pss:
                ln_apply("ml", pss, mf_f32, 2, g_ml, bt_ml, mfn_bf)
            mfnF = ag(sb_tiles(mfn_bf, 2), 2, "mfn")

            meta_f32 = mktile(loc, [128, 2, S], F32, tag="meta_f32")
            meta_bf = mktile(loc, [128, 2, S], BF16, tag="meta_bf")

            def ev_meta(mt, psum):
                t = mktile(loc, [128, S], F32, tag="meta_t")
                nc.scalar.activation(t[:], psum[:], AF.Identity,
                                     bias=b_mla[:, mt:mt + 1])
                nc.vector.tensor_add(meta_f32[:, mt, :], t[:],
                                     ao_f32[:, mt, :])
                nc.vector.tensor_copy(meta_bf[:, mt, :], meta_f32[:, mt, :])
            layer(ml_aw_d, H, NC, rhs_stream(mfnF, 16), ev_meta)
            metaF = ag(sb_tiles(meta_bf, 2), 2, "meta")

            # ============ genetic ============
            mgF = layer_ag("mg", mut_w1_d, H, NC, rhs_stream(metaF, 16),
                           b_mu1, act="gelu")
            mut_f32 = mktile(loc, [128, 2, S], F32, tag="mut_f32")
            layer(mut_w2_d, H, NC, rhs_stream(mgF, 16),
                  lambda mt, psum: nc.scalar.activation(
                      mut_f32[:, mt, :], psum[:], AF.Identity,
                      bias=b_mu2[:, mt:mt + 1]))
            gen_bf = mktile(loc, [128, 2, S], BF16, tag="gen_bf")
            for mt in range(2):
                d = mktile(loc, [128, S], F32, tag="gen_d")
                nc.vector.tensor_sub(d[:], meta_f32[:, mt, :],
                                     mut_f32[:, mt, :])
                pr = mktile(loc, [128, S], F32, tag="gen_p")
                nc.vector.scalar_tensor_tensor(pr[:], frT[:, mt, :],
                                               1.0 / KPOP, d[:],
                                               ALU.mult, ALU.mult)
                gf = mktile(loc, [128, S], F32, tag="gen_f")
                nc.vector.tensor_add(gf[:], mut_f32[:, mt, :], pr[:])
                nc.vector.tensor_copy(gen_bf[:, mt, :], gf[:])
            genF = ag(sb_tiles(gen_bf, 2), 2, "gen")

            # ============ evolution ============
            e1F = layer_ag("e1", ev_w1_d, H, 2 * NC, rhs_stream(genF, 16),
                           b_ev1, act="gelu")
            e2F = layer_ag("e2", ev_w2_d, 2 * H, 2 * NC, rhs_stream(e1F, 32),
                           b_ev2, act="gelu")
            e3F = layer_ag("e3", ev_w3_d, 2 * H, NC, rhs_stream(e2F, 32),
                           b_ev3, act="copy")
            evoF = layer_ag("evo", wao_d, H, NC, rhs_stream(e3F, 16),
                            b_ao, act="copy")

            # ============ bayes ============
            lkgF = layer_ag("lkg", lk_w1_d, H, NC, rhs_stream(evoF, 16),
                            b_lk1, act="gelu")
            lkF = layer_ag("lk", lk_w2_d, H, NC, rhs_stream(lkgF, 16),
                           b_lk2, act="copy")
            pgF = layer_ag("pg", po_w1_d, H, NC, rhs_stream(lkF, 16),
                           b_po1, act="gelu")
            postF = layer_ag("post", po_w2_d, H, NC, rhs_stream(pgF, 16),
                             b_po2, act="copy")

            # ============ integration ============
            parts = [aoF, metaF, genF, evoF, postF]
            streams = [rhs_stream(f, 16) for f in parts]

            def comb_rhs(kt):
                return streams[kt // 16](kt % 16)
            i1F = layer_ag("i1", int_w1_d, 5 * H, 2 * NC, comb_rhs, b_i1,
                           act="gelu")
            i2_f32 = mktile(loc, [128, 2, S], F32, tag="i2_f32")
            layer(int_w2_d, 2 * H, NC, rhs_stream(i1F, 32),
                  lambda mt, psum: nc.scalar.activation(
                      i2_f32[:, mt, :], psum[:], AF.Identity,
                      bias=b_i2[:, mt:mt + 1]))
            integ_bf = mktile(loc, [128, 2, S], BF16, tag="integ_bf")
            with tc.tile_pool(name="pint", bufs=1, space="PSUM") as pss:
                ln_apply("int", pss, i2_f32, 2, g_i, bt_i, integ_bf)
            integF = ag(sb_tiles(integ_bf, 2), 2, "integ")

            # ============ output projection ============
            with tc.tile_pool(name="evp", bufs=2) as evp:
                integT = mktile(xp, [128, 16, S], BF16, tag="xresid")
                nc.sync.dma_start(
                    out=integT[:],
                    in_=integF[:].rearrange("(kt p) s -> p kt s", p=128))
                NCH = (VC + 511) // 512
                for ch in range(NCH):
                    c0 = ch * 512
                    w = min(512, VC - c0)
                    wt = mktile(wp, [128, 16, w], BF16, tag="w")
                    nc.sync.dma_start(
                        out=wt[:],
                        in_=out_w_d[:, ds(c0, w)].rearrange(
                            "(kt p) n -> p kt n", p=128))
                    bb = mktile(evp, [128, w], F32, tag="ob_bc")
                    nc.sync.dma_start(
                        out=bb[:],
                        in_=out_b_d[ds(c0, w)].partition_broadcast(128))
                    for st in range(4):
                        op = mktile(ps, [128, w], F32, tag=f"mm{st}")
                        for kt in range(16):
                            nc.tensor.matmul(op[:],
                                             integT[:, kt, ts(st, 128)],
                                             wt[:, kt, :],
                                             start=(kt == 0), stop=(kt == 15))
                        osb = mktile(evp, [128, w], F32, tag="osb")
                        nc.vector.tensor_add(osb[:], op[:], bb[:])
                        nc.sync.dma_start(out=out_d[ts(st, 128), ds(c0, w)],
                                          in_=osb[:])

            if debug:
                for nm, f in [("avF", avF), ("aoF", aoF), ("g1F", g1F),
                              ("mfnF", mfnF), ("metaF", metaF), ("mgF", mgF),
                              ("genF", genF), ("e1F", e1F), ("e2F", e2F),
                              ("e3F", e3F), ("evoF", evoF), ("lkgF", lkgF),
                              ("lkF", lkF), ("pgF", pgF), ("postF", postF),
                              ("i1F", i1F), ("integF", integF)]:
                    o = nc.dram_tensor(f"dbg_{nm}", list(f.shape), BF16,
                                       kind="ExternalOutput")
                    nc.sync.dma_start(out=o[:], in_=f[:])
                o = nc.dram_tensor("dbg_frT", [128, 2, S], F32,
                                   kind="ExternalOutput")
                nc.sync.dma_start(out=o[:], in_=frT[:])

    nc.finalize()
    return nc


# ======================= host side =======================
_PROG_CACHE = {}


def _get_prog(debug=False):
    if debug not in _PROG_CACHE:
        _PROG_CACHE[debug] = build_program(debug)
    return _PROG_CACHE[debug]


def _erf(x):
    try:
        from scipy.special import erf as _e
        return _e(x)
    except Exception:
        import math
        return np.vectorize(math.erf)(np.asarray(x, np.float64)).astype(np.float32)


def _prep_inputs(inputs_embeds, crossover_u, params):
    p = {k: np.asarray(v, dtype=np.float32) for k, v in params.items()}
    x = np.asarray(inputs_embeds, np.float32).reshape(S, H)
    u = np.asarray(crossover_u, np.float32).reshape(KPOP, S, H)

    W_ao = p["ao_w1"] @ p["ao_w2"] @ p["ao_w3"] @ p["ao_w4"]
    b_ao = ((p["ao_b1"] @ p["ao_w2"] + p["ao_b2"]) @ p["ao_w3"]
            + p["ao_b3"]) @ p["ao_w4"] + p["ao_b4"]
    gb = p["pr_b1"]
    pv = (0.5 * gb * (1.0 + _erf(gb / np.sqrt(2.0)))).astype(np.float32)
    prior_vec = pv @ p["pr_w2"] + p["pr_b2"]
    po_b1_eff = prior_vec @ p["po_w1"][:H] + p["po_b1"]

    lnm = np.zeros((2 * NCORES, 2), np.float32)
    lnm[0::2, 0] = 1.0
    lnm[1::2, 1] = 1.0

    def bf16(a):
        return np.ascontiguousarray(a).astype(BF)

    shared = {
        "xT": bf16(x.T), "ln_mask": lnm,
        "qo_w1": bf16(p["qo_w1"]), "qo_w2": bf16(p["qo_w2"]),
        "qo_b1": p["qo_b1"], "qo_b2": p["qo_b2"],
    }
    cw = {
        "wq": (p["wq"], None, None), "wk": (p["wk"], None, None),
        "wv": (p["wv"], None, None), "wo": (p["wo"], None, None),
        "ml_w1": (p["ml_w1"], p["ml_b1"], "ml_b1"),
        "ml_w2": (p["ml_w2"], p["ml_b2"], "ml_b2"),
        "ml_aw": (p["ml_aw"], p["ml_ab"], "ml_ab"),
        "mut_w1": (p["mut_w1"], p["mut_b1"], "mut_b1"),
        "mut_w2": (p["mut_w2"], p["mut_b2"], "mut_b2"),
        "ev_w1": (p["ev_w1"], p["ev_b1"], "ev_b1"),
        "ev_w2": (p["ev_w2"], p["ev_b2"], "ev_b2"),
        "ev_w3": (p["ev_w3"], p["ev_b3"], "ev_b3"),
        "wao": (W_ao, b_ao, "bao"),
        "lk_w1": (p["lk_w1"], p["lk_b1"], "lk_b1"),
        "lk_w2": (p["lk_w2"], p["lk_b2"], "lk_b2"),
        "po_w1": (p["po_w1"][H:], po_b1_eff, "po_b1"),
        "po_w2": (p["po_w2"], p["po_b2"], "po_b2"),
        "int_w1": (p["int_w1"], p["int_b1"], "int_b1"),
        "int_w2": (p["int_w2"], p["int_b2"], "int_b2"),
        "out_w": (p["out_w"], p["out_b"], "out_b"),
    }
    vec_shard = {"ml_g": p["ml_g"], "ml_bt": p["ml_beta"],
                 "int_g": p["int_g"], "int_bt": p["int_beta"]}

    in_maps = []
    for c in range(NCORES):
        m = dict(shared)
        for name, (w, b, bname) in cw.items():
            ncols = w.shape[1] // NCORES
            sl = slice(c * ncols, (c + 1) * ncols)
            m[name] = bf16(w[:, sl])
            if b is not None:
                m[bname] = np.ascontiguousarray(b[sl], dtype=np.float32)
        for name, v in vec_shard.items():
            m[name] = np.ascontiguousarray(v[c * NC:(c + 1) * NC],
                                           dtype=np.float32)
        m["u"] = np.ascontiguousarray(u[:, :, c * NC:(c + 1) * NC])
        in_maps.append(m)
    return in_maps


def run(inputs_embeds, crossover_u, params, debug=False, trace=False):
    nc = _get_prog(debug)
    in_maps = _prep_inputs(inputs_embeds, crossover_u, params)
    res = run_bass_kernel_spmd(nc, in_maps, list(range(NCORES)), trace=trace)
    out = np.empty((1, S, V), np.float32)
    for c in range(NCORES):
        out[0, :, c * VC:(c + 1) * VC] = res.results[c]["out"]
    return out, res


def kernel(inputs_embeds, crossover_u, params):
    out, _ = run(inputs_embeds, crossover_u, params)
    return out


# revision 9
# speedup vs baseline: 1.3434x; 1.3434x over previous
"""Trainium2 Bass kernel for nn_ModelX_47004122088026 (8 NeuronCores).

Strategy (tensor-parallel, feature/col-split across 8 cores):
- All activations kept transposed [feature, seq] (feature on partitions).
- Every Linear is column-split: core c computes its 256 output features,
  then a bf16 AllGather rebuilds the full activation for the next layer.
- Attention is head-split (2 heads/core, head_dim=128).
- Algebraic restructuring (exact):
  * The genetic population is 100 identical copies of `meta`, so the
    selection net + top_k are no-ops: best == meta. The crossover mean
    collapses to genetic = mut + (cnt/50)*(meta-mut) with
    cnt = sum_k (u_k < 0.8), computed on-device from crossover_u.
  * The architecture_optimizer chain (4 bias-linears, no nonlinearity)
    folds into one matrix W_ao (host fold) and bias b_ao.
  * prior_network(0) contribution folds into po_b1 (host).
- Matmuls in bf16 (fp32 PSUM accumulation); softmax/LN internals fp32.
"""
import numpy as np
import ml_dtypes

import concourse.bacc as bacc
import concourse.mybir as mybir
import concourse.tile as tile
from concourse.bass import ds, ts
from concourse.bass_utils import run_bass_kernel_spmd
from concourse.masks import make_identity

F32 = mybir.dt.float32
BF16 = mybir.dt.bfloat16
AF = mybir.ActivationFunctionType
ALU = mybir.AluOpType
AX = mybir.AxisListType

NCORES = 8
S = 512            # sequence
H = 2048           # hidden
NC = H // NCORES   # 256 features per core
NH, HD = 16, 128
HPC = NH // NCORES  # 2 heads per core
V = 50000
VC = V // NCORES   # 6250 vocab cols per core
KPOP = 50
CROSS = 0.8
ISQD = float(1.0 / np.sqrt(128.0))
RG = [list(range(NCORES))]

BF = ml_dtypes.bfloat16



_TC = [0]


def mktile(pool, shape, dtype, tag, bufs=1, **kw):
    _TC[0] += 1
    return pool.tile(shape, dtype, tag=tag, bufs=bufs, name=f"{tag}_{_TC[0]}", **kw)

def build_program(debug=False):
    nc = bacc.Bacc("TRN2", target_bir_lowering=False, debug=False,
                   num_devices=NCORES)

    def din(name, shape, dt=BF16):
        return nc.dram_tensor(name, shape, dt, kind="ExternalInput")

    xT_d = din("xT", [H, S])
    u_d = din("u", [S, KPOP, NC], F32)
    lnm_d = din("ln_mask", [2 * NCORES, 2], F32)
    qo_w1_d = din("qo_w1", [HD, 2 * HD])
    qo_w2_d = din("qo_w2", [2 * HD, HD])
    qo_b1_d = din("qo_b1", [2 * HD], F32)
    qo_b2_d = din("qo_b2", [HD], F32)
    wq_d = din("wq", [H, NC]); wk_d = din("wk", [H, NC]); wv_d = din("wv", [H, NC])
    wo_d = din("wo", [H, NC])
    ml_w1_d = din("ml_w1", [H, 2 * NC]); ml_b1_d = din("ml_b1", [2 * NC], F32)
    ml_w2_d = din("ml_w2", [2 * H, NC]); ml_b2_d = din("ml_b2", [NC], F32)
    ml_g_d = din("ml_g", [NC], F32); ml_bt_d = din("ml_bt", [NC], F32)
    ml_aw_d = din("ml_aw", [H, NC]); ml_ab_d = din("ml_ab", [NC], F32)
    mut_w1_d = din("mut_w1", [H, NC]); mut_b1_d = din("mut_b1", [NC], F32)
    mut_w2_d = din("mut_w2", [H, NC]); mut_b2_d = din("mut_b2", [NC], F32)
    ev_w1_d = din("ev_w1", [H, 2 * NC]); ev_b1_d = din("ev_b1", [2 * NC], F32)
    ev_w2_d = din("ev_w2", [2 * H, 2 * NC]); ev_b2_d = din("ev_b2", [2 * NC], F32)
    ev_w3_d = din("ev_w3", [2 * H, NC]); ev_b3_d = din("ev_b3", [NC], F32)
    wao_d = din("wao", [H, NC]); bao_d = din("bao", [NC], F32)
    lk_w1_d = din("lk_w1", [H, NC]); lk_b1_d = din("lk_b1", [NC], F32)
    lk_w2_d = din("lk_w2", [H, NC]); lk_b2_d = din("lk_b2", [NC], F32)
    po_w1_d = din("po_w1", [H, NC]); po_b1_d = din("po_b1", [NC], F32)
    po_w2_d = din("po_w2", [H, NC]); po_b2_d = din("po_b2", [NC], F32)
    int_w1_d = din("int_w1", [5 * H, 2 * NC]); int_b1_d = din("int_b1", [2 * NC], F32)
    int_w2_d = din("int_w2", [2 * H, NC]); int_b2_d = din("int_b2", [NC], F32)
    int_g_d = din("int_g", [NC], F32); int_bt_d = din("int_bt", [NC], F32)
    out_w_d = din("out_w", [H, VC]); out_b_d = din("out_b", [VC], F32)

    out_d = nc.dram_tensor("out", [S, VC], F32, kind="ExternalOutput")

    with tile.TileContext(nc) as tc:
        with (
            tc.tile_pool(name="cst", bufs=1) as cst,
            tc.tile_pool(name="xp", bufs=1) as xp,
            tc.tile_pool(name="wp", bufs=2) as wp,
            tc.tile_pool(name="fp", bufs=3) as fp,
            tc.tile_pool(name="loc", bufs=1) as loc,
            tc.tile_pool(name="ps", bufs=1, space="PSUM") as ps,
            tc.tile_pool(name="dram", bufs=1, space="DRAM") as dram,
        ):
            # ---------- constants ----------
            ident_bf = mktile(cst, [128, 128], BF16, tag="ident_bf")
            make_identity(nc, ident_bf)
            ident_f32 = mktile(cst, [128, 128], F32, tag="ident_f32")
            make_identity(nc, ident_f32)
            ones_col = mktile(cst, [128, 1], F32, tag="ones_col")
            nc.vector.memset(ones_col[:], 1.0)
            ones_row = mktile(cst, [1, 128], F32, tag="ones_row")
            nc.vector.memset(ones_row[:], 1.0)
            eps_t = mktile(cst, [1, 1], F32, tag="eps")
            nc.vector.memset(eps_t[:], 1e-5)
            lnm = mktile(cst, [2 * NCORES, 2], F32, tag="lnm")
            nc.sync.dma_start(out=lnm[:], in_=lnm_d[:])

            def bias_tile(dram_t, n, tag):
                t = mktile(cst, [128, max(1, n // 128)], F32, tag=tag)
                nc.sync.dma_start(out=t[:],
                                  in_=dram_t[:].rearrange("(t p) -> p t", p=128))
                return t

            b_qo1 = bias_tile(qo_b1_d, 2 * HD, "b_qo1")
            b_qo2 = bias_tile(qo_b2_d, HD, "b_qo2")
            b_ml1 = bias_tile(ml_b1_d, 2 * NC, "b_ml1")
            b_ml2 = bias_tile(ml_b2_d, NC, "b_ml2")
            g_ml = bias_tile(ml_g_d, NC, "g_ml")
            bt_ml = bias_tile(ml_bt_d, NC, "bt_ml")
            b_mla = bias_tile(ml_ab_d, NC, "b_mla")
            b_mu1 = bias_tile(mut_b1_d, NC, "b_mu1")
            b_mu2 = bias_tile(mut_b2_d, NC, "b_mu2")
            b_ev1 = bias_tile(ev_b1_d, 2 * NC, "b_ev1")
            b_ev2 = bias_tile(ev_b2_d, 2 * NC, "b_ev2")
            b_ev3 = bias_tile(ev_b3_d, NC, "b_ev3")
            b_ao = bias_tile(bao_d, NC, "b_ao")
            b_lk1 = bias_tile(lk_b1_d, NC, "b_lk1")
            b_lk2 = bias_tile(lk_b2_d, NC, "b_lk2")
            b_po1 = bias_tile(po_b1_d, NC, "b_po1")
            b_po2 = bias_tile(po_b2_d, NC, "b_po2")
            b_i1 = bias_tile(int_b1_d, 2 * NC, "b_i1")
            b_i2 = bias_tile(int_b2_d, NC, "b_i2")
            g_i = bias_tile(int_g_d, NC, "g_i")
            bt_i = bias_tile(int_bt_d, NC, "bt_i")

            xT_sb = mktile(xp, [128, 16, S], BF16, tag="xresid")
            nc.sync.dma_start(out=xT_sb[:],
                              in_=xT_d[:].rearrange("(kt p) s -> p kt s", p=128))

            # ---------- helpers ----------
            def ag(shard_tiles, ntiles, name):
                shard_d = mktile(dram, [ntiles * 128, S], BF16, tag=f"sh_{name}")
                for i, t in enumerate(shard_tiles):
                    nc.sync.dma_start(out=shard_d[ts(i, 128), :], in_=t)
                full_d = mktile(dram, [ntiles * 128 * NCORES, S], BF16,
                                   addr_space="Shared", tag=f"fl_{name}")
                nc.gpsimd.collective_compute(
                    "AllGather", ALU.bypass, replica_groups=RG,
                    ins=[shard_d[:]], outs=[full_d[:]])
                return full_d

            def rhs_stream(full_d, kts):
                slabs = {}
                def rhs(kt):
                    s0 = (kt // 16) * 16
                    if s0 not in slabs:
                        sl = min(16, kts - s0)
                        t = mktile(fp, [128, sl, S], BF16, tag="full")
                        nc.sync.dma_start(
                            out=t[:],
                            in_=full_d[ds(s0 * 128, sl * 128), :].rearrange(
                                "(kt p) s -> p kt s", p=128))
                        slabs[s0] = t
                    return slabs[s0][:, kt - s0, :]
                return rhs

            def layer(w_d, Kdim, ncols, rhs_fn, evict_fn):
                kts, nmt = Kdim // 128, ncols // 128
                psums = [mktile(ps, [128, S], F32, tag=f"mm{mt}")
                         for mt in range(nmt)]
                for s0 in range(0, kts, 16):
                    sl = min(16, kts - s0)
                    wt = mktile(wp, [128, sl, ncols], BF16, tag="w")
                    nc.sync.dma_start(
                        out=wt[:],
                        in_=w_d[ds(s0 * 128, sl * 128), :].rearrange(
                            "(kt p) n -> p kt n", p=128))
                    for kt in range(sl):
                        rhs = rhs_fn(s0 + kt)
                        for mt in range(nmt):
                            nc.tensor.matmul(
                                psums[mt][:], wt[:, kt, ts(mt, 128)], rhs,
                                start=(s0 + kt == 0),
                                stop=(s0 + kt == kts - 1))
                for mt in range(nmt):
                    evict_fn(mt, psums[mt])

            def sb_tiles(t, nmt):
                return [t[:, mt, :] for mt in range(nmt)]

            def layer_ag(name, w_d, Kdim, ncols, rhs_fn, bias, act="copy"):
                nmt = ncols // 128
                ot = mktile(loc, [128, nmt, S], BF16, tag="oshard", bufs=2)
                fn = AF.Gelu if act == "gelu" else AF.Identity
                def ev(mt, psum):
                    nc.scalar.activation(ot[:, mt, :], psum[:], fn,
                                         bias=bias[:, mt:mt + 1])
                layer(w_d, Kdim, ncols, rhs_fn, ev)
                return ag(sb_tiles(ot, nmt), nmt, name)

            def ln_apply(name, pss, loc_f32, nmt, gam, bet, out_bf):
                sq = mktile(loc, [128, nmt, S], F32, tag="lnsq")
                for mt in range(nmt):
                    nc.scalar.activation(sq[:, mt, :], loc_f32[:, mt, :],
                                         AF.Square)
                ps1 = mktile(pss, [1, S], F32, tag="st1")
                ps2 = mktile(pss, [1, S], F32, tag="st2")
                for mt in range(nmt):
                    nc.tensor.matmul(ps1[:], ones_col[:], loc_f32[:, mt, :],
                                     start=(mt == 0), stop=(mt == nmt - 1))
                for mt in range(nmt):
                    nc.tensor.matmul(ps2[:], ones_col[:], sq[:, mt, :],
                                     start=(mt == 0), stop=(mt == nmt - 1))
                st1_sb = mktile(loc, [1, S], F32, tag="lnst1")
                st2_sb = mktile(loc, [1, S], F32, tag="lnst2")
                nc.scalar.activation(st1_sb[:], ps1[:], AF.Copy)
                nc.scalar.activation(st2_sb[:], ps2[:], AF.Copy)
                st_sh = mktile(dram, [2, S], F32, tag=f"stsh_{name}")
                nc.sync.dma_start(out=st_sh[0:1, :], in_=st1_sb[:])
                nc.sync.dma_start(out=st_sh[1:2, :], in_=st2_sb[:])
                st_fl = mktile(dram, [2 * NCORES, S], F32, addr_space="Shared",
                                  tag=f"stfl_{name}")
                nc.gpsimd.collective_compute(
                    "AllGather", ALU.bypass, replica_groups=RG,
                    ins=[st_sh[:]], outs=[st_fl[:]])
                st_all = mktile(loc, [2 * NCORES, S], F32, tag="lnsta")
                nc.sync.dma_start(out=st_all[:], in_=st_fl[:])
                psr1 = mktile(pss, [1, S], F32, tag="st1")
                psr2 = mktile(pss, [1, S], F32, tag="st2")
                nc.tensor.matmul(psr1[:], lnm[:, 0:1], st_all[:], start=True,
                                 stop=True)
                nc.tensor.matmul(psr2[:], lnm[:, 1:2], st_all[:], start=True,
                                 stop=True)
                m = mktile(loc, [1, S], F32, tag="ln_m")
                ex2 = mktile(loc, [1, S], F32, tag="ln_e")
                nc.vector.tensor_scalar_mul(m[:], psr1[:], 1.0 / H)
                nc.vector.tensor_scalar_mul(ex2[:], psr2[:], 1.0 / H)
                mm = mktile(loc, [1, S], F32, tag="ln_mm")
                nc.vector.tensor_mul(mm[:], m[:], m[:])
                var = mktile(loc, [1, S], F32, tag="ln_v")
                nc.vector.tensor_sub(var[:], ex2[:], mm[:])
                sstd = mktile(loc, [1, S], F32, tag="ln_sd")
                nc.scalar.activation(sstd[:], var[:], AF.Sqrt, bias=eps_t[:])
                rstd = mktile(loc, [1, S], F32, tag="ln_r")
                nc.vector.reciprocal(rstd[:], sstd[:])
                nm = mktile(loc, [1, S], F32, tag="ln_nm")
                nc.vector.tensor_scalar_mul(nm[:], m[:], -1.0)
                mb = mktile(loc, [1, S], F32, tag="ln_mb")
                nc.vector.tensor_mul(mb[:], nm[:], rstd[:])
                rstd_bc = mktile(pss, [128, S], F32, tag="bc1")
                mb_bc = mktile(pss, [128, S], F32, tag="bc2")
                nc.tensor.matmul(rstd_bc[:], ones_row[:], rstd[:],
                                 start=True, stop=True)
                nc.tensor.matmul(mb_bc[:], ones_row[:], mb[:],
                                 start=True, stop=True)
                for mt in range(nmt):
                    t1 = mktile(loc, [128, S], F32, tag="ln_t1")
                    nc.vector.tensor_mul(t1[:], loc_f32[:, mt, :], rstd_bc[:])
                    t2 = mktile(loc, [128, S], F32, tag="ln_t2")
                    nc.vector.tensor_add(t2[:], t1[:], mb_bc[:])
                    nc.vector.tensor_scalar(out_bf[:, mt, :], t2[:],
                                            gam[:, mt:mt + 1],
                                            bet[:, mt:mt + 1],
                                            ALU.mult, ALU.add)

            frT = mktile(loc, [128, 2, S], F32, tag="frT")
            ao_f32 = mktile(loc, [128, 2, S], F32, tag="ao_f32")
            ao_bf = mktile(loc, [128, 2, S], BF16, tag="ao_bf")

            # ============ phase: crossover-frac + attention ============
            with (
                tc.tile_pool(name="up", bufs=2) as up,
                tc.tile_pool(name="att", bufs=1) as att,
                tc.tile_pool(name="psa", bufs=2, space="PSUM") as psa,
            ):
                # crossover count, natural layout then PE-transpose
                for st in range(4):
                    acc = [mktile(up, [128, NC], F32, tag="acc_a"),
                           mktile(up, [128, NC], F32, tag="acc_b")]
                    for kc in range(2):  # k chunks of 25
                        uch = mktile(up, [128, 25, NC], F32, tag="uch")
                        nc.sync.dma_start(
                            out=uch[:],
                            in_=u_d[ts(st, 128), ds(kc * 25, 25), :])
                        for kk in range(25):
                            k = kc * 25 + kk
                            if k == 0:
                                nc.vector.tensor_scalar(
                                    acc[0][:], uch[:, kk, :], CROSS, None,
                                    ALU.is_lt)
                            else:
                                nc.vector.scalar_tensor_tensor(
                                    acc[k % 2][:], uch[:, kk, :], CROSS,
                                    acc[(k - 1) % 2][:], ALU.is_lt, ALU.add)
                    for ft in range(2):
                        tp = mktile(psa, [128, 128], F32, tag="aux")
                        nc.tensor.transpose(
                            tp[:], acc[(KPOP - 1) % 2][:, ts(ft, 128)],
                            ident_f32[:])
                        nc.scalar.activation(frT[:, ft, ts(st, 128)], tp[:],
                                             AF.Copy)

                # q/k/v projections (2 heads per core)
                qb = mktile(att, [128, HPC, S], BF16, tag="qb")
                kb = mktile(att, [128, HPC, S], BF16, tag="kb")
                vvb = mktile(att, [128, HPC, S], BF16, tag="vvb")

                def ev_bf(t):
                    def f(mt, psum):
                        nc.scalar.activation(t[:, mt, :], psum[:], AF.Copy)
                    return f
                layer(wq_d, H, NC, lambda kt: xT_sb[:, kt, :], ev_bf(qb))
                layer(wk_d, H, NC, lambda kt: xT_sb[:, kt, :], ev_bf(kb))
                layer(wv_d, H, NC, lambda kt: xT_sb[:, kt, :], ev_bf(vvb))

                vh = mktile(att, [128, HPC, 4, 128], BF16, tag="vh")
                for h in range(HPC):
                    for kt4 in range(4):
                        tp = mktile(psa, [128, 128], BF16, tag="attp")
                        nc.tensor.transpose(tp[:], vvb[:, h, ts(kt4, 128)],
                                            ident_bf[:])
                        nc.scalar.activation(vh[:, h, kt4, :], tp[:], AF.Copy)

                qo1_sb = mktile(att, [128, 2 * HD], BF16, tag="qo1")
                nc.sync.dma_start(out=qo1_sb[:], in_=qo_w1_d[:])
                qo2_sb = mktile(att, [128, 2, HD], BF16, tag="qo2")
                nc.sync.dma_start(
                    out=qo2_sb[:],
                    in_=qo_w2_d[:].rearrange("(kt p) n -> p kt n", p=128))

                def qopt(src, h, nm):
                    g = mktile(att, [128, 2, S], BF16, tag=f"qog_{nm}")
                    for mt in range(2):
                        y1 = mktile(psa, [128, S], F32, tag="aux")
                        nc.tensor.matmul(y1[:], qo1_sb[:, ts(mt, 128)],
                                         src[:, h, :], start=True, stop=True)
                        nc.scalar.activation(g[:, mt, :], y1[:], AF.Gelu,
                                             bias=b_qo1[:, mt:mt + 1])
                    y2 = mktile(psa, [128, S], F32, tag="aux")
                    for kt in range(2):
                        nc.tensor.matmul(y2[:], qo2_sb[:, kt, :], g[:, kt, :],
                                         start=(kt == 0), stop=(kt == 1))
                    o = mktile(att, [128, S], BF16, tag=f"qh_{nm}")
                    nc.scalar.activation(o[:], y2[:], AF.Identity,
                                         bias=b_qo2[:, 0:1])
                    return o

                avb = mktile(att, [128, HPC, S], BF16, tag="avb")
                for h in range(HPC):
                    qh = qopt(qb, h, f"q{h}")
                    kh = qopt(kb, h, f"k{h}")
                    attn = mktile(att, [128, 4, S], BF16, tag="attn", bufs=2)
                    for st in range(4):
                        sc = mktile(psa, [128, S], F32, tag="aux")
                        nc.tensor.matmul(sc[:], qh[:, ts(st, 128)], kh[:],
                                         start=True, stop=True)
                        rmax = mktile(att, [128, 1], F32, tag="rmax", bufs=2)
                        nc.vector.reduce_max(rmax[:], sc[:], AX.X)
                        nbias = mktile(att, [128, 1], F32, tag="nbias", bufs=2)
                        nc.vector.tensor_scalar_mul(nbias[:], rmax[:], -ISQD)
                        esc = mktile(att, [128, S], BF16, tag="esc", bufs=2)
                        rsum = mktile(att, [128, 1], F32, tag="rsum", bufs=2)
                        nc.scalar.activation(esc[:], sc[:], AF.Exp,
                                             bias=nbias[:], scale=ISQD,
                                             accum_out=rsum[:])
                        rinv = mktile(att, [128, 1], F32, tag="rinv", bufs=2)
                        nc.vector.reciprocal(rinv[:], rsum[:])
                        nc.vector.tensor_scalar_mul(attn[:, st, :], esc[:],
                                                    rinv[:])
                    attnT = mktile(att, [128, 4, S], BF16, tag="attnT", bufs=2)
                    for st in range(4):
                        for kt4 in range(4):
                            tp = mktile(psa, [128, 128], BF16, tag="attp")
                            nc.tensor.transpose(tp[:],
                                                attn[:, st, ts(kt4, 128)],
                                                ident_bf[:])
                            nc.scalar.activation(attnT[:, kt4, ts(st, 128)],
                                                 tp[:], AF.Copy)
                    av_ps = mktile(psa, [128, S], F32, tag="aux")
                    for kt4 in range(4):
                        nc.tensor.matmul(av_ps[:], vh[:, h, kt4, :],
                                         attnT[:, kt4, :],
                                         start=(kt4 == 0), stop=(kt4 == 3))
                    nc.scalar.activation(avb[:, h, :], av_ps[:], AF.Copy)

                avF = ag(sb_tiles(avb, HPC), HPC, "av")

                def ev_ao(mt, psum):
                    nc.scalar.activation(ao_f32[:, mt, :], psum[:], AF.Copy)
                    nc.vector.tensor_copy(ao_bf[:, mt, :], ao_f32[:, mt, :])
                layer(wo_d, H, NC, rhs_stream(avF, 16), ev_ao)
                aoF = ag(sb_tiles(ao_bf, 2), 2, "ao")

            # ============ meta ============
            g1F = layer_ag("g1", ml_w1_d, H, 2 * NC, rhs_stream(aoF, 16),
                           b_ml1, act="gelu")
            mf_f32 = mktile(loc, [128, 2, S], F32, tag="mf_f32")
            layer(ml_w2_d, 2 * H, NC, rhs_stream(g1F, 32),
                  lambda mt, psum: nc.scalar.activation(
                      mf_f32[:, mt, :], psum[:], AF.Identity,
                      bias=b_ml2[:, mt:mt + 1]))
            mfn_bf = mktile(loc, [128, 2, S], BF16, tag="mfn_bf")
            with tc.tile_pool(name="pml", bufs=1, space="PSUM") as pss:
                ln_apply("ml", pss, mf_f32, 2, g_ml, bt_ml, mfn_bf)
            mfnF = ag(sb_tiles(mfn_bf, 2), 2, "mfn")

            meta_f32 = mktile(loc, [128, 2, S], F32, tag="meta_f32")
            meta_bf = mktile(loc, [128, 2, S], BF16, tag="meta_bf")

            def ev_meta(mt, psum):
                t = mktile(loc, [128, S], F32, tag="meta_t")
                nc.scalar.activation(t[:], psum[:], AF.Identity,
                                     bias=b_mla[:, mt:mt + 1])
                nc.vector.tensor_add(meta_f32[:, mt, :], t[:],
                                     ao_f32[:, mt, :])
                nc.vector.tensor_copy(meta_bf[:, mt, :], meta_f32[:, mt, :])
            layer(ml_aw_d, H, NC, rhs_stream(mfnF, 16), ev_meta)
            metaF = ag(sb_tiles(meta_bf, 2), 2, "meta")

            # ============ genetic ============
            mgF = layer_ag("mg", mut_w1_d, H, NC, rhs_stream(metaF, 16),
                           b_mu1, act="gelu")
            mut_f32 = mktile(loc, [128, 2, S], F32, tag="mut_f32")
            layer(mut_w2_d, H, NC, rhs_stream(mgF, 16),
                  lambda mt, psum: nc.scalar.activation(
                      mut_f32[:, mt, :], psum[:], AF.Identity,
                      bias=b_mu2[:, mt:mt + 1]))
            gen_bf = mktile(loc, [128, 2, S], BF16, tag="gen_bf")
            for mt in range(2):
                d = mktile(loc, [128, S], F32, tag="gen_d")
                nc.vector.tensor_sub(d[:], meta_f32[:, mt, :],
                                     mut_f32[:, mt, :])
                pr = mktile(loc, [128, S], F32, tag="gen_p")
                nc.vector.scalar_tensor_tensor(pr[:], frT[:, mt, :],
                                               1.0 / KPOP, d[:],
                                               ALU.mult, ALU.mult)
                gf = mktile(loc, [128, S], F32, tag="gen_f")
                nc.vector.tensor_add(gf[:], mut_f32[:, mt, :], pr[:])
                nc.vector.tensor_copy(gen_bf[:, mt, :], gf[:])
            genF = ag(sb_tiles(gen_bf, 2), 2, "gen")

            # ============ evolution ============
            e1F = layer_ag("e1", ev_w1_d, H, 2 * NC, rhs_stream(genF, 16),
                           b_ev1, act="gelu")
            e2F = layer_ag("e2", ev_w2_d, 2 * H, 2 * NC, rhs_stream(e1F, 32),
                           b_ev2, act="gelu")
            e3F = layer_ag("e3", ev_w3_d, 2 * H, NC, rhs_stream(e2F, 32),
                           b_ev3, act="copy")
            evoF = layer_ag("evo", wao_d, H, NC, rhs_stream(e3F, 16),
                            b_ao, act="copy")

            # ============ bayes ============
            lkgF = layer_ag("lkg", lk_w1_d, H, NC, rhs_stream(evoF, 16),
                            b_lk1, act="gelu")
            lkF = layer_ag("lk", lk_w2_d, H, NC, rhs_stream(lkgF, 16),
                           b_lk2, act="copy")
            pgF = layer_ag("pg", po_w1_d, H, NC, rhs_stream(lkF, 16),
                           b_po1, act="gelu")
            postF = layer_ag("post", po_w2_d, H, NC, rhs_stream(pgF, 16),
                             b_po2, act="copy")

            # ============ integration ============
            parts = [aoF, metaF, genF, evoF, postF]
            streams = [rhs_stream(f, 16) for f in parts]

            def comb_rhs(kt):
                return streams[kt // 16](kt % 16)
            i1F = layer_ag("i1", int_w1_d, 5 * H, 2 * NC, comb_rhs, b_i1,
                           act="gelu")
            i2_f32 = mktile(loc, [128, 2, S], F32, tag="i2_f32")
            layer(int_w2_d, 2 * H, NC, rhs_stream(i1F, 32),
                  lambda mt, psum: nc.scalar.activation(
                      i2_f32[:, mt, :], psum[:], AF.Identity,
                      bias=b_i2[:, mt:mt + 1]))
            integ_bf = mktile(loc, [128, 2, S], BF16, tag="integ_bf")
            with tc.tile_pool(name="pint", bufs=1, space="PSUM") as pss:
                ln_apply("int", pss, i2_f32, 2, g_i, bt_i, integ_bf)
            integF = ag(sb_tiles(integ_bf, 2), 2, "integ")

            # ============ output projection ============
            with tc.tile_pool(name="evp", bufs=2) as evp:
                integT = mktile(xp, [128, 16, S], BF16, tag="xresid")
                nc.sync.dma_start(
                    out=integT[:],
                    in_=integF[:].rearrange("(kt p) s -> p kt s", p=128))
                NCH = (VC + 511) // 512
                for ch in range(NCH):
                    c0 = ch * 512
                    w = min(512, VC - c0)
                    wt = mktile(wp, [128, 16, w], BF16, tag="w")
                    nc.sync.dma_start(
                        out=wt[:],
                        in_=out_w_d[:, ds(c0, w)].rearrange(
                            "(kt p) n -> p kt n", p=128))
                    bb = mktile(evp, [128, w], F32, tag="ob_bc")
                    nc.sync.dma_start(
                        out=bb[:],
                        in_=out_b_d[ds(c0, w)].partition_broadcast(128))
                    for st in range(4):
                        op = mktile(ps, [128, w], F32, tag=f"mm{st}")
                        for kt in range(16):
                            nc.tensor.matmul(op[:],
                                             integT[:, kt, ts(st, 128)],
                                             wt[:, kt, :],
                                             start=(kt == 0), stop=(kt == 15))
                        osb = mktile(evp, [128, w], F32, tag="osb")
                        nc.vector.tensor_add(osb[:], op[:], bb[:])
                        nc.sync.dma_start(out=out_d[ts(st, 128), ds(c0, w)],
                                          in_=osb[:])

            if debug:
                for nm, f in [("avF", avF), ("aoF", aoF), ("g1F", g1F),
                              ("mfnF", mfnF), ("metaF", metaF), ("mgF", mgF),
                              ("genF", genF), ("e1F", e1F), ("e2F", e2F),
                              ("e3F", e3F), ("evoF", evoF), ("lkgF", lkgF),
                              ("lkF", lkF), ("pgF", pgF), ("postF", postF),
                              ("i1F", i1F), ("integF", integF)]:
                    o = nc.dram_tensor(f"dbg_{nm}", list(f.shape), BF16,
                                       kind="ExternalOutput")
                    nc.sync.dma_start(out=o[:], in_=f[:])
                o = nc.dram_tensor("dbg_frT", [128, 2, S], F32,
                                   kind="ExternalOutput")
                nc.sync.dma_start(out=o[:], in_=frT[:])

    nc.finalize()
    return nc


# ======================= host side =======================
_PROG_CACHE = {}


def _get_prog(debug=False):
    if debug not in _PROG_CACHE:
        _PROG_CACHE[debug] = build_program(debug)
    return _PROG_CACHE[debug]


def _erf(x):
    try:
        from scipy.special import erf as _e
        return _e(x)
    except Exception:
        import math
        return np.vectorize(math.erf)(np.asarray(x, np.float64)).astype(np.float32)


def _prep_inputs(inputs_embeds, crossover_u, params):
    p = {k: np.asarray(v, dtype=np.float32) for k, v in params.items()}
    x = np.asarray(inputs_embeds, np.float32).reshape(S, H)
    u = np.asarray(crossover_u, np.float32).reshape(KPOP, S, H)
    ut = np.ascontiguousarray(u.transpose(1, 0, 2))  # [S, KPOP, H]

    W_ao = p["ao_w1"] @ p["ao_w2"] @ p["ao_w3"] @ p["ao_w4"]
    b_ao = ((p["ao_b1"] @ p["ao_w2"] + p["ao_b2"]) @ p["ao_w3"]
            + p["ao_b3"]) @ p["ao_w4"] + p["ao_b4"]
    gb = p["pr_b1"]
    pv = (0.5 * gb * (1.0 + _erf(gb / np.sqrt(2.0)))).astype(np.float32)
    prior_vec = pv @ p["pr_w2"] + p["pr_b2"]
    po_b1_eff = prior_vec @ p["po_w1"][:H] + p["po_b1"]

    lnm = np.zeros((2 * NCORES, 2), np.float32)
    lnm[0::2, 0] = 1.0
    lnm[1::2, 1] = 1.0

    def bf16(a):
        return np.ascontiguousarray(a).astype(BF)

    shared = {
        "xT": bf16(x.T), "ln_mask": lnm,
        "qo_w1": bf16(p["qo_w1"]), "qo_w2": bf16(p["qo_w2"]),
        "qo_b1": p["qo_b1"], "qo_b2": p["qo_b2"],
    }
    cw = {
        "wq": (p["wq"], None, None), "wk": (p["wk"], None, None),
        "wv": (p["wv"], None, None), "wo": (p["wo"], None, None),
        "ml_w1": (p["ml_w1"], p["ml_b1"], "ml_b1"),
        "ml_w2": (p["ml_w2"], p["ml_b2"], "ml_b2"),
        "ml_aw": (p["ml_aw"], p["ml_ab"], "ml_ab"),
        "mut_w1": (p["mut_w1"], p["mut_b1"], "mut_b1"),
        "mut_w2": (p["mut_w2"], p["mut_b2"], "mut_b2"),
        "ev_w1": (p["ev_w1"], p["ev_b1"], "ev_b1"),
        "ev_w2": (p["ev_w2"], p["ev_b2"], "ev_b2"),
        "ev_w3": (p["ev_w3"], p["ev_b3"], "ev_b3"),
        "wao": (W_ao, b_ao, "bao"),
        "lk_w1": (p["lk_w1"], p["lk_b1"], "lk_b1"),
        "lk_w2": (p["lk_w2"], p["lk_b2"], "lk_b2"),
        "po_w1": (p["po_w1"][H:], po_b1_eff, "po_b1"),
        "po_w2": (p["po_w2"], p["po_b2"], "po_b2"),
        "int_w1": (p["int_w1"], p["int_b1"], "int_b1"),
        "int_w2": (p["int_w2"], p["int_b2"], "int_b2"),
        "out_w": (p["out_w"], p["out_b"], "out_b"),
    }
    vec_shard = {"ml_g": p["ml_g"], "ml_bt": p["ml_beta"],
                 "int_g": p["int_g"], "int_bt": p["int_beta"]}

    in_maps = []
    for c in range(NCORES):
        m = dict(shared)
        for name, (w, b, bname) in cw.items():
            ncols = w.shape[1] // NCORES
            sl = slice(c * ncols, (c + 1) * ncols)
            m[name] = bf16(w[:, sl])
            if b is not None:
                m[bname] = np.ascontiguousarray(b[sl], dtype=np.float32)
        for name, v in vec_shard.items():
            m[name] = np.ascontiguousarray(v[c * NC:(c + 1) * NC],
                                           dtype=np.float32)
        m["u"] = np.ascontiguousarray(ut[:, :, c * NC:(c + 1) * NC])
        in_maps.append(m)
    return in_maps


def run(inputs_embeds, crossover_u, params, debug=False, trace=False):
    nc = _get_prog(debug)
    in_maps = _prep_inputs(inputs_embeds, crossover_u, params)
    res = run_bass_kernel_spmd(nc, in_maps, list(range(NCORES)), trace=trace)
    out = np.empty((1, S, V), np.float32)
    for c in range(NCORES):
        out[0, :, c * VC:(c + 1) * VC] = res.results[c]["out"]
    return out, res


def kernel(inputs_embeds, crossover_u, params):
    out, _ = run(inputs_embeds, crossover_u, params)
    return out


# revision 10
# speedup vs baseline: 1.5040x; 1.1196x over previous
"""Trainium2 Bass kernel for nn_ModelX_47004122088026 (8 NeuronCores).

Strategy (tensor-parallel, feature/col-split across 8 cores):
- All activations kept transposed [feature, seq] (feature on partitions).
- Every Linear is column-split: core c computes its 256 output features,
  then a bf16 AllGather rebuilds the full activation for the next layer.
- Attention is head-split (2 heads/core, head_dim=128).
- Algebraic restructuring (exact):
  * The genetic population is 100 identical copies of `meta`, so the
    selection net + top_k are no-ops: best == meta. The crossover mean
    collapses to genetic = mut + (cnt/50)*(meta-mut) with
    cnt = sum_k (u_k < 0.8), computed on-device from crossover_u.
  * The architecture_optimizer chain (4 bias-linears, no nonlinearity)
    folds into one matrix W_ao (host fold) and bias b_ao.
  * prior_network(0) contribution folds into po_b1 (host).
- Matmuls in bf16 (fp32 PSUM accumulation); softmax/LN internals fp32.
"""
import numpy as np
import ml_dtypes

import concourse.bacc as bacc
import concourse.mybir as mybir
import concourse.tile as tile
from concourse.bass import ds, ts
from concourse.bass_utils import run_bass_kernel_spmd
from concourse.masks import make_identity

F32 = mybir.dt.float32
BF16 = mybir.dt.bfloat16
AF = mybir.ActivationFunctionType
ALU = mybir.AluOpType
AX = mybir.AxisListType

NCORES = 8
S = 512            # sequence
H = 2048           # hidden
NC = H // NCORES   # 256 features per core
NH, HD = 16, 128
HPC = NH // NCORES  # 2 heads per core
V = 50000
VC = V // NCORES   # 6250 vocab cols per core
KPOP = 50
CROSS = 0.8
ISQD = float(1.0 / np.sqrt(128.0))
RG = [list(range(NCORES))]

BF = ml_dtypes.bfloat16



_TC = [0]


def mktile(pool, shape, dtype, tag, bufs=1, **kw):
    _TC[0] += 1
    return pool.tile(shape, dtype, tag=tag, bufs=bufs, name=f"{tag}_{_TC[0]}", **kw)

def build_program(debug=False):
    nc = bacc.Bacc("TRN2", target_bir_lowering=False, debug=False,
                   num_devices=NCORES)

    def din(name, shape, dt=BF16):
        return nc.dram_tensor(name, shape, dt, kind="ExternalInput")

    def dw(name, K, ncols):
        # host-packed partition-major weight: [128, K//128, ncols]
        return nc.dram_tensor(name, [128, K // 128, ncols], BF16,
                              kind="ExternalInput")

    xT_d = nc.dram_tensor("xT", [128, 16, S], BF16, kind="ExternalInput")
    u_d = din("u", [S, KPOP, NC], F32)
    lnm_d = din("ln_mask", [2 * NCORES, 2], F32)
    qo_w1_d = din("qo_w1", [HD, 2 * HD])
    qo_w2_d = din("qo_w2", [2 * HD, HD])
    qo_b1_d = din("qo_b1", [2 * HD], F32)
    qo_b2_d = din("qo_b2", [HD], F32)
    wq_d = dw("wq", H, NC); wk_d = dw("wk", H, NC); wv_d = dw("wv", H, NC)
    wo_d = dw("wo", H, NC)
    ml_w1_d = dw("ml_w1", H, 2 * NC); ml_b1_d = din("ml_b1", [2 * NC], F32)
    ml_w2_d = dw("ml_w2", 2 * H, NC); ml_b2_d = din("ml_b2", [NC], F32)
    ml_g_d = din("ml_g", [NC], F32); ml_bt_d = din("ml_bt", [NC], F32)
    ml_aw_d = dw("ml_aw", H, NC); ml_ab_d = din("ml_ab", [NC], F32)
    mut_w1_d = dw("mut_w1", H, NC); mut_b1_d = din("mut_b1", [NC], F32)
    mut_w2_d = dw("mut_w2", H, NC); mut_b2_d = din("mut_b2", [NC], F32)
    ev_w1_d = dw("ev_w1", H, 2 * NC); ev_b1_d = din("ev_b1", [2 * NC], F32)
    ev_w2_d = dw("ev_w2", 2 * H, 2 * NC); ev_b2_d = din("ev_b2", [2 * NC], F32)
    ev_w3_d = dw("ev_w3", 2 * H, NC); ev_b3_d = din("ev_b3", [NC], F32)
    wao_d = dw("wao", H, NC); bao_d = din("bao", [NC], F32)
    lk_w1_d = dw("lk_w1", H, NC); lk_b1_d = din("lk_b1", [NC], F32)
    lk_w2_d = dw("lk_w2", H, NC); lk_b2_d = din("lk_b2", [NC], F32)
    po_w1_d = dw("po_w1", H, NC); po_b1_d = din("po_b1", [NC], F32)
    po_w2_d = dw("po_w2", H, NC); po_b2_d = din("po_b2", [NC], F32)
    int_w1_d = dw("int_w1", 5 * H, 2 * NC); int_b1_d = din("int_b1", [2 * NC], F32)
    int_w2_d = dw("int_w2", 2 * H, NC); int_b2_d = din("int_b2", [NC], F32)
    int_g_d = din("int_g", [NC], F32); int_bt_d = din("int_bt", [NC], F32)
    out_w_d = nc.dram_tensor("out_w", [13, 128, 16, 512], BF16, kind="ExternalInput"); out_b_d = din("out_b", [VC], F32)

    out_d = nc.dram_tensor("out", [S, VC], F32, kind="ExternalOutput")

    with tile.TileContext(nc) as tc:
        with (
            tc.tile_pool(name="cst", bufs=1) as cst,
            tc.tile_pool(name="xp", bufs=1) as xp,
            tc.tile_pool(name="wp", bufs=2) as wp,
            tc.tile_pool(name="fp", bufs=3) as fp,
            tc.tile_pool(name="loc", bufs=1) as loc,
            tc.tile_pool(name="ps", bufs=1, space="PSUM") as ps,
            tc.tile_pool(name="dram", bufs=1, space="DRAM") as dram,
        ):
            # ---------- constants ----------
            ident_bf = mktile(cst, [128, 128], BF16, tag="ident_bf")
            make_identity(nc, ident_bf)
            ident_f32 = mktile(cst, [128, 128], F32, tag="ident_f32")
            make_identity(nc, ident_f32)
            ones_col = mktile(cst, [128, 1], F32, tag="ones_col")
            nc.vector.memset(ones_col[:], 1.0)
            ones_row = mktile(cst, [1, 128], F32, tag="ones_row")
            nc.vector.memset(ones_row[:], 1.0)
            eps_t = mktile(cst, [1, 1], F32, tag="eps")
            nc.vector.memset(eps_t[:], 1e-5)
            lnm = mktile(cst, [2 * NCORES, 2], F32, tag="lnm")
            nc.sync.dma_start(out=lnm[:], in_=lnm_d[:])

            def bias_tile(dram_t, n, tag):
                t = mktile(cst, [128, max(1, n // 128)], F32, tag=tag)
                nc.sync.dma_start(out=t[:],
                                  in_=dram_t[:].rearrange("(t p) -> p t", p=128))
                return t

            b_qo1 = bias_tile(qo_b1_d, 2 * HD, "b_qo1")
            b_qo2 = bias_tile(qo_b2_d, HD, "b_qo2")
            b_ml1 = bias_tile(ml_b1_d, 2 * NC, "b_ml1")
            b_ml2 = bias_tile(ml_b2_d, NC, "b_ml2")
            g_ml = bias_tile(ml_g_d, NC, "g_ml")
            bt_ml = bias_tile(ml_bt_d, NC, "bt_ml")
            b_mla = bias_tile(ml_ab_d, NC, "b_mla")
            b_mu1 = bias_tile(mut_b1_d, NC, "b_mu1")
            b_mu2 = bias_tile(mut_b2_d, NC, "b_mu2")
            b_ev1 = bias_tile(ev_b1_d, 2 * NC, "b_ev1")
            b_ev2 = bias_tile(ev_b2_d, 2 * NC, "b_ev2")
            b_ev3 = bias_tile(ev_b3_d, NC, "b_ev3")
            b_ao = bias_tile(bao_d, NC, "b_ao")
            b_lk1 = bias_tile(lk_b1_d, NC, "b_lk1")
            b_lk2 = bias_tile(lk_b2_d, NC, "b_lk2")
            b_po1 = bias_tile(po_b1_d, NC, "b_po1")
            b_po2 = bias_tile(po_b2_d, NC, "b_po2")
            b_i1 = bias_tile(int_b1_d, 2 * NC, "b_i1")
            b_i2 = bias_tile(int_b2_d, NC, "b_i2")
            g_i = bias_tile(int_g_d, NC, "g_i")
            bt_i = bias_tile(int_bt_d, NC, "bt_i")

            xT_sb = mktile(xp, [128, 16, S], BF16, tag="xresid")
            for q in range(4):
                nc.sync.dma_start(out=xT_sb[:, ts(q, 4), :],
                                  in_=xT_d[:, ts(q, 4), :])

            # ---------- helpers ----------
            def ag(shard_tiles, ntiles, name):
                shard_d = mktile(dram, [ntiles * 128, S], BF16, tag=f"sh_{name}")
                for i, t in enumerate(shard_tiles):
                    nc.sync.dma_start(out=shard_d[ts(i, 128), :], in_=t)
                full_d = mktile(dram, [ntiles * 128 * NCORES, S], BF16,
                                   addr_space="Shared", tag=f"fl_{name}")
                nc.gpsimd.collective_compute(
                    "AllGather", ALU.bypass, replica_groups=RG,
                    ins=[shard_d[:]], outs=[full_d[:]])
                return full_d

            def rhs_stream(full_d, kts):
                slabs = {}
                def rhs(kt):
                    s0 = (kt // 16) * 16
                    if s0 not in slabs:
                        sl = min(16, kts - s0)
                        t = mktile(fp, [128, sl, S], BF16, tag="full")
                        for q0 in range(0, sl, 4):
                            qn = min(4, sl - q0)
                            nc.sync.dma_start(
                                out=t[:, ds(q0, qn), :],
                                in_=full_d[ds((s0 + q0) * 128, qn * 128), :]
                                .rearrange("(kt p) s -> p kt s", p=128))
                        slabs[s0] = t
                    return slabs[s0][:, kt - s0, :]
                return rhs

            def layer(w_d, Kdim, ncols, rhs_fn, evict_fn):
                kts, nmt = Kdim // 128, ncols // 128
                psums = [mktile(ps, [128, S], F32, tag=f"mm{mt}")
                         for mt in range(nmt)]
                for s0 in range(0, kts, 16):
                    sl = min(16, kts - s0)
                    wt = mktile(wp, [128, sl, ncols], BF16, tag="w")
                    for q0 in range(0, sl, 4):
                        qn = min(4, sl - q0)
                        nc.sync.dma_start(
                            out=wt[:, ds(q0, qn), :],
                            in_=w_d[:, ds(s0 + q0, qn), :])
                    for kt in range(sl):
                        rhs = rhs_fn(s0 + kt)
                        for mt in range(nmt):
                            nc.tensor.matmul(
                                psums[mt][:], wt[:, kt, ts(mt, 128)], rhs,
                                start=(s0 + kt == 0),
                                stop=(s0 + kt == kts - 1))
                for mt in range(nmt):
                    evict_fn(mt, psums[mt])

            def sb_tiles(t, nmt):
                return [t[:, mt, :] for mt in range(nmt)]

            def layer_ag(name, w_d, Kdim, ncols, rhs_fn, bias, act="copy"):
                nmt = ncols // 128
                ot = mktile(loc, [128, nmt, S], BF16, tag="oshard", bufs=2)
                fn = AF.Gelu if act == "gelu" else AF.Identity
                def ev(mt, psum):
                    nc.scalar.activation(ot[:, mt, :], psum[:], fn,
                                         bias=bias[:, mt:mt + 1])
                layer(w_d, Kdim, ncols, rhs_fn, ev)
                return ag(sb_tiles(ot, nmt), nmt, name)

            def ln_apply(name, pss, loc_f32, nmt, gam, bet, out_bf):
                sq = mktile(loc, [128, nmt, S], F32, tag="lnsq")
                for mt in range(nmt):
                    nc.scalar.activation(sq[:, mt, :], loc_f32[:, mt, :],
                                         AF.Square)
                ps1 = mktile(pss, [1, S], F32, tag="st1")
                ps2 = mktile(pss, [1, S], F32, tag="st2")
                for mt in range(nmt):
                    nc.tensor.matmul(ps1[:], ones_col[:], loc_f32[:, mt, :],
                                     start=(mt == 0), stop=(mt == nmt - 1))
                for mt in range(nmt):
                    nc.tensor.matmul(ps2[:], ones_col[:], sq[:, mt, :],
                                     start=(mt == 0), stop=(mt == nmt - 1))
                st1_sb = mktile(loc, [1, S], F32, tag="lnst1")
                st2_sb = mktile(loc, [1, S], F32, tag="lnst2")
                nc.scalar.activation(st1_sb[:], ps1[:], AF.Copy)
                nc.scalar.activation(st2_sb[:], ps2[:], AF.Copy)
                st_sh = mktile(dram, [2, S], F32, tag=f"stsh_{name}")
                nc.sync.dma_start(out=st_sh[0:1, :], in_=st1_sb[:])
                nc.sync.dma_start(out=st_sh[1:2, :], in_=st2_sb[:])
                st_fl = mktile(dram, [2 * NCORES, S], F32, addr_space="Shared",
                                  tag=f"stfl_{name}")
                nc.gpsimd.collective_compute(
                    "AllGather", ALU.bypass, replica_groups=RG,
                    ins=[st_sh[:]], outs=[st_fl[:]])
                st_all = mktile(loc, [2 * NCORES, S], F32, tag="lnsta")
                nc.sync.dma_start(out=st_all[:], in_=st_fl[:])
                psr1 = mktile(pss, [1, S], F32, tag="st1")
                psr2 = mktile(pss, [1, S], F32, tag="st2")
                nc.tensor.matmul(psr1[:], lnm[:, 0:1], st_all[:], start=True,
                                 stop=True)
                nc.tensor.matmul(psr2[:], lnm[:, 1:2], st_all[:], start=True,
                                 stop=True)
                m = mktile(loc, [1, S], F32, tag="ln_m")
                ex2 = mktile(loc, [1, S], F32, tag="ln_e")
                nc.vector.tensor_scalar_mul(m[:], psr1[:], 1.0 / H)
                nc.vector.tensor_scalar_mul(ex2[:], psr2[:], 1.0 / H)
                mm = mktile(loc, [1, S], F32, tag="ln_mm")
                nc.vector.tensor_mul(mm[:], m[:], m[:])
                var = mktile(loc, [1, S], F32, tag="ln_v")
                nc.vector.tensor_sub(var[:], ex2[:], mm[:])
                sstd = mktile(loc, [1, S], F32, tag="ln_sd")
                nc.scalar.activation(sstd[:], var[:], AF.Sqrt, bias=eps_t[:])
                rstd = mktile(loc, [1, S], F32, tag="ln_r")
                nc.vector.reciprocal(rstd[:], sstd[:])
                nm = mktile(loc, [1, S], F32, tag="ln_nm")
                nc.vector.tensor_scalar_mul(nm[:], m[:], -1.0)
                mb = mktile(loc, [1, S], F32, tag="ln_mb")
                nc.vector.tensor_mul(mb[:], nm[:], rstd[:])
                rstd_bc = mktile(pss, [128, S], F32, tag="bc1")
                mb_bc = mktile(pss, [128, S], F32, tag="bc2")
                nc.tensor.matmul(rstd_bc[:], ones_row[:], rstd[:],
                                 start=True, stop=True)
                nc.tensor.matmul(mb_bc[:], ones_row[:], mb[:],
                                 start=True, stop=True)
                for mt in range(nmt):
                    t1 = mktile(loc, [128, S], F32, tag="ln_t1")
                    nc.vector.tensor_mul(t1[:], loc_f32[:, mt, :], rstd_bc[:])
                    t2 = mktile(loc, [128, S], F32, tag="ln_t2")
                    nc.vector.tensor_add(t2[:], t1[:], mb_bc[:])
                    nc.vector.tensor_scalar(out_bf[:, mt, :], t2[:],
                                            gam[:, mt:mt + 1],
                                            bet[:, mt:mt + 1],
                                            ALU.mult, ALU.add)

            frT = mktile(loc, [128, 2, S], F32, tag="frT")
            ao_f32 = mktile(loc, [128, 2, S], F32, tag="ao_f32")
            ao_bf = mktile(loc, [128, 2, S], BF16, tag="ao_bf")

            # ============ phase: crossover-frac + attention ============
            with (
                tc.tile_pool(name="up", bufs=2) as up,
                tc.tile_pool(name="att", bufs=1) as att,
                tc.tile_pool(name="psa", bufs=2, space="PSUM") as psa,
            ):
                # crossover count, natural layout then PE-transpose
                for st in range(4):
                    acc = [mktile(up, [128, NC], F32, tag="acc_a"),
                           mktile(up, [128, NC], F32, tag="acc_b")]
                    for kc in range(2):  # k chunks of 25
                        uch = mktile(up, [128, 25, NC], F32, tag="uch")
                        nc.sync.dma_start(
                            out=uch[:],
                            in_=u_d[ts(st, 128), ds(kc * 25, 25), :])
                        for kk in range(25):
                            k = kc * 25 + kk
                            if k == 0:
                                nc.vector.tensor_scalar(
                                    acc[0][:], uch[:, kk, :], CROSS, None,
                                    ALU.is_lt)
                            else:
                                nc.vector.scalar_tensor_tensor(
                                    acc[k % 2][:], uch[:, kk, :], CROSS,
                                    acc[(k - 1) % 2][:], ALU.is_lt, ALU.add)
                    for ft in range(2):
                        tp = mktile(psa, [128, 128], F32, tag="aux")
                        nc.tensor.transpose(
                            tp[:], acc[(KPOP - 1) % 2][:, ts(ft, 128)],
                            ident_f32[:])
                        nc.scalar.activation(frT[:, ft, ts(st, 128)], tp[:],
                                             AF.Copy)

                # q/k/v projections (2 heads per core)
                qb = mktile(att, [128, HPC, S], BF16, tag="qb")
                kb = mktile(att, [128, HPC, S], BF16, tag="kb")
                vvb = mktile(att, [128, HPC, S], BF16, tag="vvb")

                def ev_bf(t):
                    def f(mt, psum):
                        nc.scalar.activation(t[:, mt, :], psum[:], AF.Copy)
                    return f
                layer(wq_d, H, NC, lambda kt: xT_sb[:, kt, :], ev_bf(qb))
                layer(wk_d, H, NC, lambda kt: xT_sb[:, kt, :], ev_bf(kb))
                layer(wv_d, H, NC, lambda kt: xT_sb[:, kt, :], ev_bf(vvb))

                vh = mktile(att, [128, HPC, 4, 128], BF16, tag="vh")
                for h in range(HPC):
                    for kt4 in range(4):
                        tp = mktile(psa, [128, 128], BF16, tag="attp")
                        nc.tensor.transpose(tp[:], vvb[:, h, ts(kt4, 128)],
                                            ident_bf[:])
                        nc.scalar.activation(vh[:, h, kt4, :], tp[:], AF.Copy)

                qo1_sb = mktile(att, [128, 2 * HD], BF16, tag="qo1")
                nc.sync.dma_start(out=qo1_sb[:], in_=qo_w1_d[:])
                qo2_sb = mktile(att, [128, 2, HD], BF16, tag="qo2")
                nc.sync.dma_start(
                    out=qo2_sb[:],
                    in_=qo_w2_d[:].rearrange("(kt p) n -> p kt n", p=128))

                def qopt(src, h, nm):
                    g = mktile(att, [128, 2, S], BF16, tag=f"qog_{nm}")
                    for mt in range(2):
                        y1 = mktile(psa, [128, S], F32, tag="aux")
                        nc.tensor.matmul(y1[:], qo1_sb[:, ts(mt, 128)],
                                         src[:, h, :], start=True, stop=True)
                        nc.scalar.activation(g[:, mt, :], y1[:], AF.Gelu,
                                             bias=b_qo1[:, mt:mt + 1])
                    y2 = mktile(psa, [128, S], F32, tag="aux")
                    for kt in range(2):
                        nc.tensor.matmul(y2[:], qo2_sb[:, kt, :], g[:, kt, :],
                                         start=(kt == 0), stop=(kt == 1))
                    o = mktile(att, [128, S], BF16, tag=f"qh_{nm}")
                    nc.scalar.activation(o[:], y2[:], AF.Identity,
                                         bias=b_qo2[:, 0:1])
                    return o

                avb = mktile(att, [128, HPC, S], BF16, tag="avb")
                for h in range(HPC):
                    qh = qopt(qb, h, f"q{h}")
                    kh = qopt(kb, h, f"k{h}")
                    attn = mktile(att, [128, 4, S], BF16, tag="attn", bufs=2)
                    for st in range(4):
                        sc = mktile(psa, [128, S], F32, tag="aux")
                        nc.tensor.matmul(sc[:], qh[:, ts(st, 128)], kh[:],
                                         start=True, stop=True)
                        rmax = mktile(att, [128, 1], F32, tag="rmax", bufs=2)
                        nc.vector.reduce_max(rmax[:], sc[:], AX.X)
                        nbias = mktile(att, [128, 1], F32, tag="nbias", bufs=2)
                        nc.vector.tensor_scalar_mul(nbias[:], rmax[:], -ISQD)
                        esc = mktile(att, [128, S], BF16, tag="esc", bufs=2)
                        rsum = mktile(att, [128, 1], F32, tag="rsum", bufs=2)
                        nc.scalar.activation(esc[:], sc[:], AF.Exp,
                                             bias=nbias[:], scale=ISQD,
                                             accum_out=rsum[:])
                        rinv = mktile(att, [128, 1], F32, tag="rinv", bufs=2)
                        nc.vector.reciprocal(rinv[:], rsum[:])
                        nc.vector.tensor_scalar_mul(attn[:, st, :], esc[:],
                                                    rinv[:])
                    attnT = mktile(att, [128, 4, S], BF16, tag="attnT", bufs=2)
                    for st in range(4):
                        for kt4 in range(4):
                            tp = mktile(psa, [128, 128], BF16, tag="attp")
                            nc.tensor.transpose(tp[:],
                                                attn[:, st, ts(kt4, 128)],
                                                ident_bf[:])
                            nc.scalar.activation(attnT[:, kt4, ts(st, 128)],
                                                 tp[:], AF.Copy)
                    av_ps = mktile(psa, [128, S], F32, tag="aux")
                    for kt4 in range(4):
                        nc.tensor.matmul(av_ps[:], vh[:, h, kt4, :],
                                         attnT[:, kt4, :],
                                         start=(kt4 == 0), stop=(kt4 == 3))
                    nc.scalar.activation(avb[:, h, :], av_ps[:], AF.Copy)

                avF = ag(sb_tiles(avb, HPC), HPC, "av")

                def ev_ao(mt, psum):
                    nc.scalar.activation(ao_f32[:, mt, :], psum[:], AF.Copy)
                    nc.vector.tensor_copy(ao_bf[:, mt, :], ao_f32[:, mt, :])
                layer(wo_d, H, NC, rhs_stream(avF, 16), ev_ao)
                aoF = ag(sb_tiles(ao_bf, 2), 2, "ao")

            # ============ meta ============
            g1F = layer_ag("g1", ml_w1_d, H, 2 * NC, rhs_stream(aoF, 16),
                           b_ml1, act="gelu")
            mf_f32 = mktile(loc, [128, 2, S], F32, tag="mf_f32")
            layer(ml_w2_d, 2 * H, NC, rhs_stream(g1F, 32),
                  lambda mt, psum: nc.scalar.activation(
                      mf_f32[:, mt, :], psum[:], AF.Identity,
                      bias=b_ml2[:, mt:mt + 1]))
            mfn_bf = mktile(loc, [128, 2, S], BF16, tag="mfn_bf")
            with tc.tile_pool(name="pml", bufs=1, space="PSUM") as pss:
                ln_apply("ml", pss, mf_f32, 2, g_ml, bt_ml, mfn_bf)
            mfnF = ag(sb_tiles(mfn_bf, 2), 2, "mfn")

            meta_f32 = mktile(loc, [128, 2, S], F32, tag="meta_f32")
            meta_bf = mktile(loc, [128, 2, S], BF16, tag="meta_bf")

            def ev_meta(mt, psum):
                t = mktile(loc, [128, S], F32, tag="meta_t")
                nc.scalar.activation(t[:], psum[:], AF.Identity,
                                     bias=b_mla[:, mt:mt + 1])
                nc.vector.tensor_add(meta_f32[:, mt, :], t[:],
                                     ao_f32[:, mt, :])
                nc.vector.tensor_copy(meta_bf[:, mt, :], meta_f32[:, mt, :])
            layer(ml_aw_d, H, NC, rhs_stream(mfnF, 16), ev_meta)
            metaF = ag(sb_tiles(meta_bf, 2), 2, "meta")

            # ============ genetic ============
            mgF = layer_ag("mg", mut_w1_d, H, NC, rhs_stream(metaF, 16),
                           b_mu1, act="gelu")
            mut_f32 = mktile(loc, [128, 2, S], F32, tag="mut_f32")
            layer(mut_w2_d, H, NC, rhs_stream(mgF, 16),
                  lambda mt, psum: nc.scalar.activation(
                      mut_f32[:, mt, :], psum[:], AF.Identity,
                      bias=b_mu2[:, mt:mt + 1]))
            gen_bf = mktile(loc, [128, 2, S], BF16, tag="gen_bf")
            for mt in range(2):
                d = mktile(loc, [128, S], F32, tag="gen_d")
                nc.vector.tensor_sub(d[:], meta_f32[:, mt, :],
                                     mut_f32[:, mt, :])
                pr = mktile(loc, [128, S], F32, tag="gen_p")
                nc.vector.scalar_tensor_tensor(pr[:], frT[:, mt, :],
                                               1.0 / KPOP, d[:],
                                               ALU.mult, ALU.mult)
                gf = mktile(loc, [128, S], F32, tag="gen_f")
                nc.vector.tensor_add(gf[:], mut_f32[:, mt, :], pr[:])
                nc.vector.tensor_copy(gen_bf[:, mt, :], gf[:])
            genF = ag(sb_tiles(gen_bf, 2), 2, "gen")

            # ============ evolution ============
            e1F = layer_ag("e1", ev_w1_d, H, 2 * NC, rhs_stream(genF, 16),
                           b_ev1, act="gelu")
            e2F = layer_ag("e2", ev_w2_d, 2 * H, 2 * NC, rhs_stream(e1F, 32),
                           b_ev2, act="gelu")
            e3F = layer_ag("e3", ev_w3_d, 2 * H, NC, rhs_stream(e2F, 32),
                           b_ev3, act="copy")
            evoF = layer_ag("evo", wao_d, H, NC, rhs_stream(e3F, 16),
                            b_ao, act="copy")

            # ============ bayes ============
            lkgF = layer_ag("lkg", lk_w1_d, H, NC, rhs_stream(evoF, 16),
                            b_lk1, act="gelu")
            lkF = layer_ag("lk", lk_w2_d, H, NC, rhs_stream(lkgF, 16),
                           b_lk2, act="copy")
            pgF = layer_ag("pg", po_w1_d, H, NC, rhs_stream(lkF, 16),
                           b_po1, act="gelu")
            postF = layer_ag("post", po_w2_d, H, NC, rhs_stream(pgF, 16),
                             b_po2, act="copy")

            # ============ integration ============
            parts = [aoF, metaF, genF, evoF, postF]
            streams = [rhs_stream(f, 16) for f in parts]

            def comb_rhs(kt):
                return streams[kt // 16](kt % 16)
            i1F = layer_ag("i1", int_w1_d, 5 * H, 2 * NC, comb_rhs, b_i1,
                           act="gelu")
            i2_f32 = mktile(loc, [128, 2, S], F32, tag="i2_f32")
            layer(int_w2_d, 2 * H, NC, rhs_stream(i1F, 32),
                  lambda mt, psum: nc.scalar.activation(
                      i2_f32[:, mt, :], psum[:], AF.Identity,
                      bias=b_i2[:, mt:mt + 1]))
            integ_bf = mktile(loc, [128, 2, S], BF16, tag="integ_bf")
            with tc.tile_pool(name="pint", bufs=1, space="PSUM") as pss:
                ln_apply("int", pss, i2_f32, 2, g_i, bt_i, integ_bf)
            integF = ag(sb_tiles(integ_bf, 2), 2, "integ")

            # ============ output projection ============
            with tc.tile_pool(name="evp", bufs=2) as evp:
                integT = mktile(xp, [128, 16, S], BF16, tag="xresid")
                for q in range(4):
                    nc.sync.dma_start(
                        out=integT[:, ts(q, 4), :],
                        in_=integF[ds(q * 512, 512), :].rearrange(
                            "(kt p) s -> p kt s", p=128))
                NCH = (VC + 511) // 512
                for ch in range(NCH):
                    c0 = ch * 512
                    w = min(512, VC - c0)
                    wt = mktile(wp, [128, 16, 512], BF16, tag="w")
                    for q in range(2):
                        nc.sync.dma_start(out=wt[:, ts(q, 8), :],
                                          in_=out_w_d[ch, :, ts(q, 8), :])
                    bb = mktile(evp, [128, w], F32, tag="ob_bc")
                    nc.sync.dma_start(
                        out=bb[:],
                        in_=out_b_d[ds(c0, w)].partition_broadcast(128))
                    for st in range(4):
                        op = mktile(ps, [128, w], F32, tag=f"mm{st}")
                        for kt in range(16):
                            nc.tensor.matmul(op[:],
                                             integT[:, kt, ts(st, 128)],
                                             wt[:, kt, :w],
                                             start=(kt == 0), stop=(kt == 15))
                        osb = mktile(evp, [128, w], F32, tag="osb")
                        nc.vector.tensor_add(osb[:], op[:], bb[:])
                        nc.sync.dma_start(out=out_d[ts(st, 128), ds(c0, w)],
                                          in_=osb[:])

            if debug:
                for nm, f in [("avF", avF), ("aoF", aoF), ("g1F", g1F),
                              ("mfnF", mfnF), ("metaF", metaF), ("mgF", mgF),
                              ("genF", genF), ("e1F", e1F), ("e2F", e2F),
                              ("e3F", e3F), ("evoF", evoF), ("lkgF", lkgF),
                              ("lkF", lkF), ("pgF", pgF), ("postF", postF),
                              ("i1F", i1F), ("integF", integF)]:
                    o = nc.dram_tensor(f"dbg_{nm}", list(f.shape), BF16,
                                       kind="ExternalOutput")
                    nc.sync.dma_start(out=o[:], in_=f[:])
                o = nc.dram_tensor("dbg_frT", [128, 2, S], F32,
                                   kind="ExternalOutput")
                nc.sync.dma_start(out=o[:], in_=frT[:])

    nc.finalize()
    return nc


# ======================= host side =======================
_PROG_CACHE = {}


def _get_prog(debug=False):
    if debug not in _PROG_CACHE:
        _PROG_CACHE[debug] = build_program(debug)
    return _PROG_CACHE[debug]


def _erf(x):
    try:
        from scipy.special import erf as _e
        return _e(x)
    except Exception:
        import math
        return np.vectorize(math.erf)(np.asarray(x, np.float64)).astype(np.float32)


def _prep_inputs(inputs_embeds, crossover_u, params):
    p = {k: np.asarray(v, dtype=np.float32) for k, v in params.items()}
    x = np.asarray(inputs_embeds, np.float32).reshape(S, H)
    u = np.asarray(crossover_u, np.float32).reshape(KPOP, S, H)
    ut = np.ascontiguousarray(u.transpose(1, 0, 2))  # [S, KPOP, H]

    W_ao = p["ao_w1"] @ p["ao_w2"] @ p["ao_w3"] @ p["ao_w4"]
    b_ao = ((p["ao_b1"] @ p["ao_w2"] + p["ao_b2"]) @ p["ao_w3"]
            + p["ao_b3"]) @ p["ao_w4"] + p["ao_b4"]
    gb = p["pr_b1"]
    pv = (0.5 * gb * (1.0 + _erf(gb / np.sqrt(2.0)))).astype(np.float32)
    prior_vec = pv @ p["pr_w2"] + p["pr_b2"]
    po_b1_eff = prior_vec @ p["po_w1"][:H] + p["po_b1"]

    lnm = np.zeros((2 * NCORES, 2), np.float32)
    lnm[0::2, 0] = 1.0
    lnm[1::2, 1] = 1.0

    def bf16(a):
        return np.ascontiguousarray(a).astype(BF)

    def pack_w(a):
        K, n = a.shape
        return np.ascontiguousarray(
            a.reshape(K // 128, 128, n).transpose(1, 0, 2)).astype(BF)

    shared = {
        "xT": pack_w(np.ascontiguousarray(x.T)), "ln_mask": lnm,
        "qo_w1": bf16(p["qo_w1"]), "qo_w2": bf16(p["qo_w2"]),
        "qo_b1": p["qo_b1"], "qo_b2": p["qo_b2"],
    }
    cw = {
        "wq": (p["wq"], None, None), "wk": (p["wk"], None, None),
        "wv": (p["wv"], None, None), "wo": (p["wo"], None, None),
        "ml_w1": (p["ml_w1"], p["ml_b1"], "ml_b1"),
        "ml_w2": (p["ml_w2"], p["ml_b2"], "ml_b2"),
        "ml_aw": (p["ml_aw"], p["ml_ab"], "ml_ab"),
        "mut_w1": (p["mut_w1"], p["mut_b1"], "mut_b1"),
        "mut_w2": (p["mut_w2"], p["mut_b2"], "mut_b2"),
        "ev_w1": (p["ev_w1"], p["ev_b1"], "ev_b1"),
        "ev_w2": (p["ev_w2"], p["ev_b2"], "ev_b2"),
        "ev_w3": (p["ev_w3"], p["ev_b3"], "ev_b3"),
        "wao": (W_ao, b_ao, "bao"),
        "lk_w1": (p["lk_w1"], p["lk_b1"], "lk_b1"),
        "lk_w2": (p["lk_w2"], p["lk_b2"], "lk_b2"),
        "po_w1": (p["po_w1"][H:], po_b1_eff, "po_b1"),
        "po_w2": (p["po_w2"], p["po_b2"], "po_b2"),
        "int_w1": (p["int_w1"], p["int_b1"], "int_b1"),
        "int_w2": (p["int_w2"], p["int_b2"], "int_b2"),
        "out_w": (p["out_w"], p["out_b"], "out_b"),
    }
    vec_shard = {"ml_g": p["ml_g"], "ml_bt": p["ml_beta"],
                 "int_g": p["int_g"], "int_bt": p["int_beta"]}

    in_maps = []
    for c in range(NCORES):
        m = dict(shared)
        for name, (w, b, bname) in cw.items():
            ncols = w.shape[1] // NCORES
            sl = slice(c * ncols, (c + 1) * ncols)
            if name == "out_w":
                wc = w[:, sl].astype(BF)  # [2048, 6250]
                pk = np.zeros((13, 128, 16, 512), BF)
                for ch in range(13):
                    c0 = ch * 512
                    wch = min(512, VC - c0)
                    blk = wc[:, c0:c0 + wch].reshape(16, 128, wch)
                    pk[ch, :, :, :wch] = blk.transpose(1, 0, 2)
                m[name] = pk
            else:
                m[name] = pack_w(w[:, sl])
            if b is not None:
                m[bname] = np.ascontiguousarray(b[sl], dtype=np.float32)
        for name, v in vec_shard.items():
            m[name] = np.ascontiguousarray(v[c * NC:(c + 1) * NC],
                                           dtype=np.float32)
        m["u"] = np.ascontiguousarray(ut[:, :, c * NC:(c + 1) * NC])
        in_maps.append(m)
    return in_maps


def run(inputs_embeds, crossover_u, params, debug=False, trace=False):
    nc = _get_prog(debug)
    in_maps = _prep_inputs(inputs_embeds, crossover_u, params)
    res = run_bass_kernel_spmd(nc, in_maps, list(range(NCORES)), trace=trace)
    out = np.empty((1, S, V), np.float32)
    for c in range(NCORES):
        out[0, :, c * VC:(c + 1) * VC] = res.results[c]["out"]
    return out, res


def kernel(inputs_embeds, crossover_u, params):
    out, _ = run(inputs_embeds, crossover_u, params)
    return out
